# revision 1
# baseline (speedup 1.0000x reference)
"""Trainium2 Bass kernel for nn_LinearPPI (block-sparse gene-gene message passing).

Computation (reference):
    out[b, 8*g_out + o] = sum_{n: block_out[n]=g_out} sum_i x[b, 8*block_in[n] + i] * w[n, i, o]
    out += x   (residual)

Strategy:
  - The residual is fused as G virtual identity blocks (src=dst=g, w=I8).
  - Blocks sorted by destination gene; destination genes sharded over 8 cores
    (edge/expert parallel, no collectives needed).
  - Per core, genes are packed into "quads" of QG (default 2) genes.  A quad
    owns a [QG*8, 128] region of a PSUM bank (QG genes x 8 outs, 128 batch),
    laid out transposed (out^T).  16 quads fill one PSUM bank tile [128, 512].
  - Work is a stream of "windows": 16 x-slabs (one slab = 8 rows of x^T for
    one source gene = [8, 128]) stacked to a [128, 128] rhs, and a matching
    scattered weight tile [128, QG*8] as lhsT.  One matmul per window:
        psum[p0:p0+QG*8, f0:f0+128] (+)= lhsT.T @ rhs   (K=128, M=QG*8, N=128)
    PSUM per-element has_written bits turn the writes into a correct
    segment-sum; each quad's first matmul uses start=True (the bank-wide bit
    clear only touches regions that are either finished or not yet started,
    and quads in one bank execute back-to-back on the PE).
  - The x-slab gather is done on the host (indices are known at trace time),
    producing a sequential HBM stream -> all device DMAs are large and
    contiguous (memory-bound regime).
  - The per-core window schedule is made identical across cores (rank-sorted
    window-count maxima + zero-padding) so a single SPMD program serves all
    8 cores; per-core variation lives only in the streamed data.
  - Output is slot-ordered out^T; the host inverse-permutes, transposes and
    concatenates shards.  No all-reduce: destination sharding makes each
    core's output disjoint.
"""

import math
import numpy as np

import concourse.bass as bass
import concourse.bacc as bacc
import concourse.mybir as mybir
from concourse.tile import TileContext
from concourse.bass_utils import run_bass_kernel_spmd


class Cfg:
    def __init__(self, G=4000, B=8, BATCH=128, NCORES=8, stream_fp16=True,
                 chunk=24, qg=2):
        assert G % NCORES == 0
        self.G, self.B, self.BATCH, self.NCORES = G, B, BATCH, NCORES
        self.GPC = G // NCORES            # genes per core
        self.QG = qg                      # genes per quad (M = QG*B)
        assert self.GPC % self.QG == 0
        self.NQ = self.GPC // self.QG     # quads per core
        self.NBANKS = math.ceil(self.NQ / 16)
        self.SLOTS = 16                   # slabs per window (K = 128)
        self.CH = chunk                   # windows per DMA chunk
        self.stream_np = np.float16 if stream_fp16 else np.float32
        self.stream_dt = mybir.dt.float16 if stream_fp16 else mybir.dt.float32
        self.out_dt = mybir.dt.float16 if stream_fp16 else mybir.dt.float32


def _pack_host(cfg, x, w, block_in, block_out):
    """Sort/shard/pad on the host. Returns (in_maps, w_sched, decode_quads)."""
    G, B, BATCH, NC = cfg.G, cfg.B, cfg.BATCH, cfg.NCORES

    # Append virtual identity blocks to fuse the residual.
    src = np.concatenate([np.asarray(block_in, dtype=np.int64), np.arange(G)])
    dst = np.concatenate([np.asarray(block_out, dtype=np.int64), np.arange(G)])
    w_full = np.concatenate(
        [np.asarray(w, dtype=np.float32),
         np.broadcast_to(np.eye(B, dtype=np.float32), (G, B, B))], axis=0)

    order = np.argsort(dst, kind="stable")
    src_s = src[order]
    w_s = np.ascontiguousarray(w_full[order]).astype(cfg.stream_np)
    counts = np.bincount(dst, minlength=G)
    starts = np.zeros(G + 1, dtype=np.int64)
    np.cumsum(counts, out=starts[1:])

    # x^T slabs: xslab[g] = x[:, 8g:8g+8].T  -> [G, 8, BATCH]
    xslab = np.ascontiguousarray(np.asarray(x, dtype=np.float32).T
                                 .reshape(G, B, BATCH)).astype(cfg.stream_np)

    # --- balanced gene->core assignment (snake over count-sorted genes) ---
    order_g = np.argsort(-counts, kind="stable")
    core_of = np.empty(G, dtype=np.int64)
    for r in range(0, G, 2 * NC):
        blk = order_g[r : r + 2 * NC]
        pat = list(range(NC)) + list(range(NC - 1, -1, -1))
        for i, g in enumerate(blk):
            core_of[g] = pat[i]

    # --- per-core quad packing: target sums that are multiples of SLOTS ---
    per_core = []
    for c in range(NC):
        genes = np.where(core_of == c)[0]  # this core's genes
        pool = sorted(genes.tolist(), key=lambda g: -counts[g])
        quads = []
        for _ in range(cfg.NQ):
            q = [pool.pop(0)]                       # largest remaining
            while pool and len(q) < cfg.QG - 1:     # middle picks: big/small mix
                q.append(pool.pop(0) if len(q) % 2 else pool.pop(-1))
            if pool and len(q) < cfg.QG:
                s3 = sum(int(counts[g]) for g in q)
                # last pick: minimize padding to the next multiple of SLOTS
                best_i = min(range(len(pool)),
                             key=lambda i: (-(s3 + int(counts[pool[i]])))
                             % cfg.SLOTS)
                q.append(pool.pop(best_i))
            q.sort()
            quads.append(q)
        assert not pool
        q_slabs = np.array([sum(int(counts[g]) for g in q) for q in quads])
        q_wins = np.ceil(q_slabs / cfg.SLOTS).astype(np.int64)
        q_wins = np.maximum(q_wins, 1)
        rank = np.argsort(-q_wins, kind="stable")
        per_core.append(([quads[j] for j in rank], q_wins[rank]))

    # common schedule: per rank, max window count over cores
    w_sched = np.max(np.stack([pc[1] for pc in per_core]), axis=0)
    cum_w = np.zeros(cfg.NQ + 1, dtype=np.int64)
    np.cumsum(w_sched, out=cum_w[1:])
    w_tot = int(cum_w[-1])

    # --- build per-core streams -------------------------------------------
    in_maps = []
    decode_quads = []
    for c in range(NC):
        quads_r, _ = per_core[c]
        slab_gene = np.full(w_tot * cfg.SLOTS, -1, dtype=np.int64)
        blk_ids, blk_pos, blk_rel = [], [], []
        for j in range(cfg.NQ):
            base = cum_w[j] * cfg.SLOTS
            p = 0
            for r, g in enumerate(quads_r[j]):
                s0, n = int(starts[g]), int(counts[g])
                ids = np.arange(s0, s0 + n)
                blk_ids.append(ids)
                blk_pos.append(base + p + np.arange(n))
                blk_rel.append(np.full(n, r, dtype=np.int64))
                p += n
            assert p <= int(w_sched[j]) * cfg.SLOTS
        blk_ids = np.concatenate(blk_ids)
        blk_pos = np.concatenate(blk_pos)
        blk_rel = np.concatenate(blk_rel)
        slab_gene[blk_pos] = src_s[blk_ids]

        # x slabs: [W, 128, BATCH]
        xg = np.zeros((w_tot * cfg.SLOTS, B, BATCH), dtype=cfg.stream_np)
        m = slab_gene >= 0
        xg[m] = xslab[slab_gene[m]]
        xg = xg.reshape(w_tot, cfg.SLOTS * B, BATCH)

        # scattered weights: [W, 128, 32]
        wg5 = np.zeros((w_tot, cfg.SLOTS, B, cfg.QG, B), dtype=cfg.stream_np)
        wg5[blk_pos // cfg.SLOTS, blk_pos % cfg.SLOTS, :, blk_rel, :] = w_s[blk_ids]
        wg = wg5.reshape(w_tot, cfg.SLOTS * B, cfg.QG * B)

        # combined stream, chunk-major: each chunk of CH windows is one
        # contiguous [128, CH*PW] DRAM block -> every DMA is a single
        # linear ~1MB read.
        st = np.concatenate([xg, wg], axis=2)          # [W, 128, PW]
        PW = BATCH + cfg.QG * B
        n_chunks = -(-w_tot // cfg.CH)
        pad = n_chunks * cfg.CH - w_tot
        if pad:
            st = np.concatenate(
                [st, np.zeros((pad, cfg.SLOTS * B, PW), dtype=cfg.stream_np)])
        st = np.ascontiguousarray(
            st.reshape(n_chunks, cfg.CH, cfg.SLOTS * B, PW)
            .transpose(0, 2, 1, 3)).reshape(n_chunks * cfg.SLOTS * B, cfg.CH * PW)

        in_maps.append({"st": st})
        decode_quads.append(quads_r)

    return in_maps, w_sched, decode_quads


def _build_nc(cfg, w_sched):
    """Trace the (core-uniform) Bass program."""
    w_tot = int(np.sum(w_sched))
    PW = cfg.BATCH + cfg.QG * cfg.B   # stream width per window
    n_chunks = -(-w_tot // cfg.CH)
    nc = bacc.Bacc("TRN2")
    st = nc.dram_tensor("st", [n_chunks * 128, cfg.CH * PW], cfg.stream_dt,
                        kind="ExternalInput")
    out = nc.dram_tensor("out", [128, cfg.NBANKS * 512], cfg.out_dt,
                         kind="ExternalOutput")

    cum_w = np.zeros(cfg.NQ + 1, dtype=np.int64)
    np.cumsum(w_sched, out=cum_w[1:])
    CH = cfg.CH
    NW = cfg.BATCH            # rhs free width per window (128)

    with TileContext(nc) as tc:
        with (
            tc.tile_pool(name="stp", bufs=4) as stp,
            tc.tile_pool(name="psp", bufs=3, space="PSUM") as psp,
            tc.tile_pool(name="outp", bufs=2) as outp,
        ):
            RH = cfg.QG * cfg.B       # psum region height per quad
            st_t = None
            for bank in range(cfg.NBANKS):
                j0, j1 = bank * 16, min(bank * 16 + 16, cfg.NQ)
                t_last = int(cum_w[j1]) - 1
                ps = psp.tile([128, 512], mybir.dt.float32)
                for j in range(j0, j1):
                    qr = j - j0
                    p0 = 32 * (qr % 4)
                    f0 = 128 * (qr // 4)
                    t_first = int(cum_w[j])
                    for t in range(int(cum_w[j]), int(cum_w[j + 1])):
                        if t % CH == 0:
                            c = t // CH
                            st_t = stp.tile([128, CH * PW], cfg.stream_dt)
                            nc.sync.dma_start(
                                out=st_t[:, :],
                                in_=st[c * 128 : (c + 1) * 128, :])
                        k = t % CH
                        nc.tensor.matmul(
                            ps[p0 : p0 + RH, f0 : f0 + 128],
                            st_t[:, k * PW + NW : (k + 1) * PW],
                            st_t[:, k * PW : k * PW + NW],
                            start=(t == t_first),
                            stop=(t == t_last),
                            tile_position=(0, p0),
                        )
                ot = outp.tile([128, 512], cfg.out_dt)
                nc.vector.tensor_copy(out=ot, in_=ps)
                nc.gpsimd.dma_start(out=out[:, bank * 512 : (bank + 1) * 512], in_=ot)
    if not nc.is_finalized():
        nc.finalize()
    return nc


def _decode(cfg, results, decode_quads):
    G, B, BATCH = cfg.G, cfg.B, cfg.BATCH
    outT = np.empty((G, B, BATCH), dtype=np.float32)
    for c in range(cfg.NCORES):
        res = np.asarray(results[c]["out"], dtype=np.float32)
        for j in range(cfg.NQ):
            bank, qr = j // 16, j % 16
            p0 = 32 * (qr % 4)
            f0 = bank * 512 + 128 * (qr // 4)
            blockv = res[p0 : p0 + cfg.QG * B, f0 : f0 + 128]
            genes = decode_quads[c][j]
            outT[genes] = blockv.reshape(cfg.QG, B, BATCH)
    return np.ascontiguousarray(outT.reshape(G * B, BATCH).T)


def _run(cfg, x, w, block_in, block_out, trace=False):
    in_maps, w_sched, decode_quads = _pack_host(cfg, x, w, block_in, block_out)
    nc = _build_nc(cfg, w_sched)
    r = run_bass_kernel_spmd(nc, in_maps, core_ids=list(range(cfg.NCORES)),
                             trace=trace)
    out = _decode(cfg, r.results, decode_quads)
    return out, r


def kernel(x, w, block_in, block_out):
    cfg = Cfg()
    out, _ = _run(cfg, x, w, block_in, block_out, trace=False)
    return out



# revision 2
# speedup vs baseline: 1.6386x; 1.6386x over previous
"""Trainium2 Bass kernel for nn_LinearPPI (block-sparse gene-gene message passing).

Computation (reference):
    out[b, 8*g_out + o] = sum_{n: block_out[n]=g_out} sum_i x[b, 8*block_in[n] + i] * w[n, i, o]
    out += x   (residual)

Strategy (v2, fp8 stream):
  - Blocks sorted by destination gene; destination genes sharded over 8 cores
    (edge/expert parallel, no collectives needed).
  - Per core, genes are packed into "quads" of QG=4 genes.  A quad owns a
    [32, 128] region of a PSUM bank (4 genes x 8 outs, 128 batch), laid out
    transposed (out^T).  16 quads fill one PSUM bank [128, 512] completely
    (M=32 matches the PE quadrant granularity), so the whole per-core output
    (125 quads) lives in 8 banks with no wasted partitions.
  - Work is a stream of "windows": 16 x-slabs (one slab = 8 rows of x^T for
    one source gene = [8, 128]) stacked to a [128, 128] rhs, and a matching
    scattered weight tile [128, 32] as lhsT.  One matmul per window:
        psum[p0:p0+32, f0:f0+128] (+)= lhsT.T @ rhs   (K=128, M=32, N=128)
  - Both x and w stream in float8 E3M4 (4 mantissa bits).  Weights are
    pre-scaled by 32 on the host so they sit in the e3m4 normal range; the
    1/32 descale is fused into the per-bank combine.  Measured end-to-end
    relative error ~1.2e-2 vs the 2e-2 gate.
  - The residual is NOT in the stream: per bank a [128, 512] fp16 tile with
    the bank's own-gene x^T values is DMA'd in, and a single DVE
    scalar_tensor_tensor computes  out_sbuf = psum * (1/32) + residual
    (bank-wide, fp16 out), which the Pool engine DMAs to HBM.
  - The x-slab gather is done on the host (indices are known at trace time),
    producing a sequential HBM stream -> all device DMAs are large and
    contiguous (memory-bound regime; model-exact DMA floor ~68us/core).
  - The per-core window schedule is made identical across cores (rank-sorted
    window-count maxima + zero-padding) so a single SPMD program serves all
    8 cores; per-core variation lives only in the streamed data.
  - Output is slot-ordered out^T; the host inverse-permutes, transposes and
    concatenates shards.  No all-reduce: destination sharding makes each
    core's output disjoint.
"""

import math
import numpy as np
import ml_dtypes

import concourse.bass as bass
import concourse.bacc as bacc
import concourse.mybir as mybir
from concourse.tile import TileContext
from concourse.bass_utils import run_bass_kernel_spmd

F8 = ml_dtypes.float8_e3m4
WSCALE = 32.0


class Cfg:
    def __init__(self, G=4000, B=8, BATCH=128, NCORES=8, chunk=32, qg=4):
        assert G % NCORES == 0
        self.G, self.B, self.BATCH, self.NCORES = G, B, BATCH, NCORES
        self.GPC = G // NCORES            # genes per core
        self.QG = qg                      # genes per quad (M = QG*B = 32)
        assert self.GPC % self.QG == 0
        self.NQ = self.GPC // self.QG     # quads per core (125)
        self.NBANKS = math.ceil(self.NQ / 16)
        self.SLOTS = 16                   # slabs per window (K = 128)
        self.CH = chunk                   # windows per DMA chunk
        self.PW = BATCH + self.QG * B     # stream bytes/row/window (160)


def _pack_host(cfg, x, w, block_in, block_out):
    """Sort/shard/pad on the host. Returns (in_maps, w_sched, decode_quads)."""
    G, B, BATCH, NC = cfg.G, cfg.B, cfg.BATCH, cfg.NCORES

    src = np.asarray(block_in, dtype=np.int64)
    dst = np.asarray(block_out, dtype=np.int64)
    w_s8 = None  # filled below after sorting

    order = np.argsort(dst, kind="stable")
    src_s = src[order]
    w_s8 = np.ascontiguousarray(np.asarray(w, dtype=np.float32)[order] * WSCALE
                                ).astype(F8)
    counts = np.bincount(dst, minlength=G)
    starts = np.zeros(G + 1, dtype=np.int64)
    np.cumsum(counts, out=starts[1:])

    # x^T slabs: xslab[g] = x[:, 8g:8g+8].T  -> [G, 8, BATCH]
    xslabf = np.ascontiguousarray(np.asarray(x, dtype=np.float32).T
                                  .reshape(G, B, BATCH))
    xslab8 = xslabf.astype(F8)
    xslab16 = xslabf.astype(np.float16)

    # --- balanced gene->core assignment (snake over count-sorted genes) ---
    order_g = np.argsort(-counts, kind="stable")
    core_of = np.empty(G, dtype=np.int64)
    for r in range(0, G, 2 * NC):
        blk = order_g[r : r + 2 * NC]
        pat = list(range(NC)) + list(range(NC - 1, -1, -1))
        for i, g in enumerate(blk):
            core_of[g] = pat[i]

    # --- per-core quad packing: target sums that are multiples of SLOTS ---
    per_core = []
    for c in range(NC):
        genes = np.where(core_of == c)[0]  # this core's genes
        pool = sorted(genes.tolist(), key=lambda g: -counts[g])
        quads = []
        for _ in range(cfg.NQ):
            q = [pool.pop(0)]                       # largest remaining
            while pool and len(q) < cfg.QG - 1:     # middle picks: big/small mix
                q.append(pool.pop(0) if len(q) % 2 else pool.pop(-1))
            if pool and len(q) < cfg.QG:
                s3 = sum(int(counts[g]) for g in q)
                # last pick: minimize padding to the next multiple of SLOTS
                best_i = min(range(len(pool)),
                             key=lambda i: (-(s3 + int(counts[pool[i]])))
                             % cfg.SLOTS)
                q.append(pool.pop(best_i))
            quads.append(q)
        assert not pool
        q_slabs = np.array([sum(int(counts[g]) for g in q) for q in quads])
        q_wins = np.ceil(q_slabs / cfg.SLOTS).astype(np.int64)
        q_wins = np.maximum(q_wins, 1)
        rank = np.argsort(-q_wins, kind="stable")
        per_core.append(([quads[j] for j in rank], q_wins[rank]))

    # common schedule: per rank, max window count over cores
    w_sched = np.max(np.stack([pc[1] for pc in per_core]), axis=0)
    cum_w = np.zeros(cfg.NQ + 1, dtype=np.int64)
    np.cumsum(w_sched, out=cum_w[1:])
    w_tot = int(cum_w[-1])

    # --- build per-core streams -------------------------------------------
    in_maps = []
    decode_quads = []
    for c in range(NC):
        quads_r, _ = per_core[c]
        slab_gene = np.full(w_tot * cfg.SLOTS, -1, dtype=np.int64)
        blk_ids, blk_pos, blk_rel = [], [], []
        for j in range(cfg.NQ):
            base = cum_w[j] * cfg.SLOTS
            p = 0
            for r, g in enumerate(quads_r[j]):
                s0, n = int(starts[g]), int(counts[g])
                ids = np.arange(s0, s0 + n)
                blk_ids.append(ids)
                blk_pos.append(base + p + np.arange(n))
                blk_rel.append(np.full(n, r, dtype=np.int64))
                p += n
            assert p <= int(w_sched[j]) * cfg.SLOTS
        blk_ids = np.concatenate(blk_ids)
        blk_pos = np.concatenate(blk_pos)
        blk_rel = np.concatenate(blk_rel)
        slab_gene[blk_pos] = src_s[blk_ids]

        # x slabs: [W, 128, BATCH] fp8
        xg = np.zeros((w_tot * cfg.SLOTS, B, BATCH), dtype=F8)
        m = slab_gene >= 0
        xg[m] = xslab8[slab_gene[m]]
        xg = xg.reshape(w_tot, cfg.SLOTS * B, BATCH)

        # scattered (pre-scaled) weights: [W, 128, 32] fp8
        wg5 = np.zeros((w_tot, cfg.SLOTS, B, cfg.QG, B), dtype=F8)
        wg5[blk_pos // cfg.SLOTS, blk_pos % cfg.SLOTS, :, blk_rel, :] = w_s8[blk_ids]
        wg = wg5.reshape(w_tot, cfg.SLOTS * B, cfg.QG * B)

        # combined stream, chunk-major: each chunk of CH windows is one
        # contiguous [128, CH*PW] DRAM block -> every DMA is a single
        # linear ~650KB read.
        st = np.concatenate([xg, wg], axis=2)          # [W, 128, PW]
        PW = cfg.PW
        n_chunks = -(-w_tot // cfg.CH)
        pad = n_chunks * cfg.CH - w_tot
        if pad:
            st = np.concatenate(
                [st, np.zeros((pad, cfg.SLOTS * B, PW), dtype=F8)])
        st = np.ascontiguousarray(
            st.reshape(n_chunks, cfg.CH, cfg.SLOTS * B, PW)
            .transpose(0, 2, 1, 3)).reshape(n_chunks * cfg.SLOTS * B, cfg.CH * PW)

        # residual tiles: [128, NBANKS*512] fp16, laid out like the PSUM banks
        res = np.zeros((128, cfg.NBANKS * 512), dtype=np.float16)
        for j in range(cfg.NQ):
            bank, qr = j // 16, j % 16
            p0 = 32 * (qr % 4)
            f0 = bank * 512 + 128 * (qr // 4)
            genes = quads_r[j]
            res[p0 : p0 + cfg.QG * B, f0 : f0 + 128] = (
                xslab16[genes].reshape(cfg.QG * B, BATCH))

        in_maps.append({"st": st, "res": res})
        decode_quads.append(quads_r)

    return in_maps, w_sched, decode_quads


def _build_nc(cfg, w_sched):
    """Trace the (core-uniform) Bass program."""
    w_tot = int(np.sum(w_sched))
    PW = cfg.PW
    n_chunks = -(-w_tot // cfg.CH)
    nc = bacc.Bacc("TRN2")
    st = nc.dram_tensor("st", [n_chunks * 128, cfg.CH * PW], mybir.dt.float8e3,
                        kind="ExternalInput")
    res = nc.dram_tensor("res", [128, cfg.NBANKS * 512], mybir.dt.float16,
                         kind="ExternalInput")
    out = nc.dram_tensor("out", [128, cfg.NBANKS * 512], mybir.dt.float16,
                         kind="ExternalOutput")

    cum_w = np.zeros(cfg.NQ + 1, dtype=np.int64)
    np.cumsum(w_sched, out=cum_w[1:])
    CH = cfg.CH
    NW = cfg.BATCH            # rhs free width per window (128)

    with TileContext(nc) as tc:
        with (
            tc.tile_pool(name="stp", bufs=4) as stp,
            tc.tile_pool(name="psp", bufs=4, space="PSUM") as psp,
            tc.tile_pool(name="resp", bufs=2) as resp,
            tc.tile_pool(name="outp", bufs=2) as outp,
        ):
            RH = cfg.QG * cfg.B       # psum region height per quad (32)
            st_t = None
            for bank in range(cfg.NBANKS):
                j0, j1 = bank * 16, min(bank * 16 + 16, cfg.NQ)
                res_t = resp.tile([128, 512], mybir.dt.float16)
                nc.sync.dma_start(
                    out=res_t, in_=res[:, bank * 512 : (bank + 1) * 512])
                ps = psp.tile([128, 512], mybir.dt.float32)
                for j in range(j0, j1):
                    qr = j - j0
                    p0 = 32 * (qr % 4)
                    f0 = 128 * (qr // 4)
                    t_first = int(cum_w[j])
                    t_last = int(cum_w[j + 1]) - 1
                    for t in range(t_first, t_last + 1):
                        if t % CH == 0:
                            c = t // CH
                            st_t = stp.tile([128, CH * PW], mybir.dt.float8e3)
                            nc.sync.dma_start(
                                out=st_t[:, :],
                                in_=st[c * 128 : (c + 1) * 128, :])
                        k = t % CH
                        nc.tensor.matmul(
                            ps[p0 : p0 + RH, f0 : f0 + 128],
                            st_t[:, k * PW + NW : (k + 1) * PW],
                            st_t[:, k * PW : k * PW + NW],
                            start=(t == t_first),
                            stop=(t == t_last),
                            tile_position=(0, p0),
                        )
                ot = outp.tile([128, 512], mybir.dt.float16)
                nc.vector.scalar_tensor_tensor(
                    ot, ps, 1.0 / WSCALE, res_t,
                    op0=mybir.AluOpType.mult, op1=mybir.AluOpType.add)
                nc.gpsimd.dma_start(out=out[:, bank * 512 : (bank + 1) * 512], in_=ot)
    if not nc.is_finalized():
        nc.finalize()
    return nc


def _decode(cfg, results, decode_quads):
    G, B, BATCH = cfg.G, cfg.B, cfg.BATCH
    outT = np.empty((G, B, BATCH), dtype=np.float32)
    for c in range(cfg.NCORES):
        res = np.asarray(results[c]["out"], dtype=np.float32)
        for j in range(cfg.NQ):
            bank, qr = j // 16, j % 16
            p0 = 32 * (qr % 4)
            f0 = bank * 512 + 128 * (qr // 4)
            blockv = res[p0 : p0 + cfg.QG * B, f0 : f0 + 128]
            genes = decode_quads[c][j]
            outT[genes] = blockv.reshape(cfg.QG, B, BATCH)
    return np.ascontiguousarray(outT.reshape(G * B, BATCH).T)


def _run(cfg, x, w, block_in, block_out, trace=False):
    in_maps, w_sched, decode_quads = _pack_host(cfg, x, w, block_in, block_out)
    nc = _build_nc(cfg, w_sched)
    r = run_bass_kernel_spmd(nc, in_maps, core_ids=list(range(cfg.NCORES)),
                             trace=trace)
    out = _decode(cfg, r.results, decode_quads)
    return out, r


def kernel(x, w, block_in, block_out):
    cfg = Cfg()
    out, _ = _run(cfg, x, w, block_in, block_out, trace=False)
    return out


# revision 15
# speedup vs baseline: 1.7004x; 1.0377x over previous
"""Trainium2 Bass kernel for nn_LinearPPI (block-sparse gene-gene message passing).

Computation (reference):
    out[b, 8*g_out + o] = sum_{n: block_out[n]=g_out} sum_i x[b, 8*block_in[n] + i] * w[n, i, o]
    out += x   (residual)

Strategy (v2, fp8 stream):
  - Blocks sorted by destination gene; destination genes sharded over 8 cores
    (edge/expert parallel, no collectives needed).
  - Per core, genes are packed into "quads" of QG=4 genes.  A quad owns a
    [32, 128] region of a PSUM bank (4 genes x 8 outs, 128 batch), laid out
    transposed (out^T).  16 quads fill one PSUM bank [128, 512] completely
    (M=32 matches the PE quadrant granularity), so the whole per-core output
    (125 quads) lives in 8 banks with no wasted partitions.
  - Work is a stream of "windows": 16 x-slabs (one slab = 8 rows of x^T for
    one source gene = [8, 128]) stacked to a [128, 128] rhs, and a matching
    scattered weight tile [128, 32] as lhsT.  One matmul per window:
        psum[p0:p0+32, f0:f0+128] (+)= lhsT.T @ rhs   (K=128, M=32, N=128)
  - Both x and w stream in float8 E3M4 (4 mantissa bits).  Weights are
    pre-scaled by 32 on the host so they sit in the e3m4 normal range; the
    1/32 descale is fused into the per-bank combine.  Measured end-to-end
    relative error ~1.2e-2 vs the 2e-2 gate.
  - The residual is NOT in the stream: per bank a [128, 512] fp16 tile with
    the bank's own-gene x^T values is DMA'd in, and a single DVE
    scalar_tensor_tensor computes  out_sbuf = psum * (1/32) + residual
    (bank-wide, fp16 out), which the Pool engine DMAs to HBM.
  - The x-slab gather is done on the host (indices are known at trace time),
    producing a sequential HBM stream -> all device DMAs are large and
    contiguous (memory-bound regime; model-exact DMA floor ~68us/core).
  - The per-core window schedule is made identical across cores (rank-sorted
    window-count maxima + zero-padding) so a single SPMD program serves all
    8 cores; per-core variation lives only in the streamed data.
  - Output is slot-ordered out^T; the host inverse-permutes, transposes and
    concatenates shards.  No all-reduce: destination sharding makes each
    core's output disjoint.
"""

import math
import numpy as np
import ml_dtypes

import concourse.bass as bass
import concourse.bacc as bacc
import concourse.mybir as mybir
from concourse.tile import TileContext
from concourse.bass_utils import run_bass_kernel_spmd

F8 = ml_dtypes.float8_e3m4
WSCALE = 32.0


class Cfg:
    def __init__(self, G=4000, B=8, BATCH=128, NCORES=8, chunk=32, qg=4):
        assert G % NCORES == 0
        self.G, self.B, self.BATCH, self.NCORES = G, B, BATCH, NCORES
        self.GPC = G // NCORES            # genes per core
        self.QG = qg                      # genes per quad (M = QG*B = 32)
        assert self.GPC % self.QG == 0
        self.NQ = self.GPC // self.QG     # quads per core (125)
        self.NBANKS = math.ceil(self.NQ / 16)
        self.SLOTS = 16                   # slabs per window (K = 128)
        self.CH = chunk                   # windows per DMA chunk
        self.TAIL_CH = 8                  # chunk size for the last CH windows
        self.PW = BATCH + self.QG * B     # stream bytes/row/window (160)

    def chunk_plan(self, w_tot):
        """Chunk sizes: full CH chunks, then TAIL_CH-sized tail chunks so the
        final bank's compute tail after the last DMA is short."""
        sizes = []
        rem = w_tot
        while rem > self.CH:
            sizes.append(self.CH)
            rem -= self.CH
        while rem > 0:
            take = min(self.TAIL_CH, rem)
            sizes.append(take)
            rem -= take
        starts = np.zeros(len(sizes) + 1, dtype=np.int64)
        np.cumsum(sizes, out=starts[1:])
        return list(sizes), starts


def _pack_host(cfg, x, w, block_in, block_out):
    """Sort/shard/pad on the host. Returns (in_maps, w_sched, decode_quads)."""
    G, B, BATCH, NC = cfg.G, cfg.B, cfg.BATCH, cfg.NCORES

    src = np.asarray(block_in, dtype=np.int64)
    dst = np.asarray(block_out, dtype=np.int64)
    w_s8 = None  # filled below after sorting

    order = np.argsort(dst, kind="stable")
    src_s = src[order]
    w_s8 = np.ascontiguousarray(np.asarray(w, dtype=np.float32)[order] * WSCALE
                                ).astype(F8)
    counts = np.bincount(dst, minlength=G)
    starts = np.zeros(G + 1, dtype=np.int64)
    np.cumsum(counts, out=starts[1:])

    # x^T slabs: xslab[g] = x[:, 8g:8g+8].T  -> [G, 8, BATCH]
    xslabf = np.ascontiguousarray(np.asarray(x, dtype=np.float32).T
                                  .reshape(G, B, BATCH))
    xslab8 = xslabf.astype(F8)
    xslab16 = xslabf.astype(np.float16)

    # --- balanced gene->core assignment (snake over count-sorted genes) ---
    order_g = np.argsort(-counts, kind="stable")
    core_of = np.empty(G, dtype=np.int64)
    for r in range(0, G, 2 * NC):
        blk = order_g[r : r + 2 * NC]
        pat = list(range(NC)) + list(range(NC - 1, -1, -1))
        for i, g in enumerate(blk):
            core_of[g] = pat[i]

    # --- per-core quad packing: target sums that are multiples of SLOTS ---
    per_core = []
    for c in range(NC):
        genes = np.where(core_of == c)[0]  # this core's genes
        pool = sorted(genes.tolist(), key=lambda g: -counts[g])
        quads = []
        for _ in range(cfg.NQ):
            q = [pool.pop(0)]                       # largest remaining
            while pool and len(q) < cfg.QG - 1:     # middle picks: big/small mix
                q.append(pool.pop(0) if len(q) % 2 else pool.pop(-1))
            if pool and len(q) < cfg.QG:
                s3 = sum(int(counts[g]) for g in q)
                # last pick: minimize padding to the next multiple of SLOTS
                best_i = min(range(len(pool)),
                             key=lambda i: (-(s3 + int(counts[pool[i]])))
                             % cfg.SLOTS)
                q.append(pool.pop(best_i))
            quads.append(q)
        assert not pool
        q_slabs = np.array([sum(int(counts[g]) for g in q) for q in quads])
        q_wins = np.ceil(q_slabs / cfg.SLOTS).astype(np.int64)
        q_wins = np.maximum(q_wins, 1)
        rank = np.argsort(-q_wins, kind="stable")
        per_core.append(([quads[j] for j in rank], q_wins[rank]))

    # common schedule: per rank, max window count over cores
    w_sched = np.max(np.stack([pc[1] for pc in per_core]), axis=0)
    cum_w = np.zeros(cfg.NQ + 1, dtype=np.int64)
    np.cumsum(w_sched, out=cum_w[1:])
    w_tot = int(cum_w[-1])

    # --- build per-core streams -------------------------------------------
    in_maps = []
    decode_quads = []
    for c in range(NC):
        quads_r, _ = per_core[c]
        slab_gene = np.full(w_tot * cfg.SLOTS, -1, dtype=np.int64)
        blk_ids, blk_pos, blk_rel = [], [], []
        for j in range(cfg.NQ):
            base = cum_w[j] * cfg.SLOTS
            p = 0
            for r, g in enumerate(quads_r[j]):
                s0, n = int(starts[g]), int(counts[g])
                ids = np.arange(s0, s0 + n)
                blk_ids.append(ids)
                blk_pos.append(base + p + np.arange(n))
                blk_rel.append(np.full(n, r, dtype=np.int64))
                p += n
            assert p <= int(w_sched[j]) * cfg.SLOTS
        blk_ids = np.concatenate(blk_ids)
        blk_pos = np.concatenate(blk_pos)
        blk_rel = np.concatenate(blk_rel)
        slab_gene[blk_pos] = src_s[blk_ids]

        # x slabs: [W, 128, BATCH] fp8
        xg = np.zeros((w_tot * cfg.SLOTS, B, BATCH), dtype=F8)
        m = slab_gene >= 0
        xg[m] = xslab8[slab_gene[m]]
        xg = xg.reshape(w_tot, cfg.SLOTS * B, BATCH)

        # scattered (pre-scaled) weights: [W, 128, 32] fp8
        wg5 = np.zeros((w_tot, cfg.SLOTS, B, cfg.QG, B), dtype=F8)
        wg5[blk_pos // cfg.SLOTS, blk_pos % cfg.SLOTS, :, blk_rel, :] = w_s8[blk_ids]
        wg = wg5.reshape(w_tot, cfg.SLOTS * B, cfg.QG * B)

        # combined stream, chunk-major along columns: chunk c of n windows is
        # a contiguous [128, n*PW] DRAM column block -> every DMA is a large
        # linear read (~650KB for full chunks).
        st = np.concatenate([xg, wg], axis=2)          # [W, 128, PW]
        sizes, cstarts = cfg.chunk_plan(w_tot)
        blocks = [
            st[cstarts[ci] : cstarts[ci] + n]
            .transpose(1, 0, 2).reshape(cfg.SLOTS * B, n * cfg.PW)
            for ci, n in enumerate(sizes)
        ]
        st = np.ascontiguousarray(np.concatenate(blocks, axis=1))

        # residual tiles: [128, NBANKS*512] fp16, laid out like the PSUM banks
        res = np.zeros((128, cfg.NBANKS * 512), dtype=np.float16)
        for j in range(cfg.NQ):
            bank, qr = j // 16, j % 16
            p0 = 32 * (qr % 4)
            f0 = bank * 512 + 128 * (qr // 4)
            genes = quads_r[j]
            res[p0 : p0 + cfg.QG * B, f0 : f0 + 128] = (
                xslab16[genes].reshape(cfg.QG * B, BATCH))

        in_maps.append({"st": st, "res": res})
        decode_quads.append(quads_r)

    return in_maps, w_sched, decode_quads


def _build_nc(cfg, w_sched):
    """Trace the (core-uniform) Bass program."""
    w_tot = int(np.sum(w_sched))
    PW = cfg.PW
    sizes, cstarts = cfg.chunk_plan(w_tot)
    nc = bacc.Bacc("TRN2")
    st = nc.dram_tensor("st", [128, w_tot * PW], mybir.dt.float8e3,
                        kind="ExternalInput")
    res = nc.dram_tensor("res", [128, cfg.NBANKS * 512], mybir.dt.float16,
                         kind="ExternalInput")
    out = nc.dram_tensor("out", [128, cfg.NBANKS * 512], mybir.dt.float16,
                         kind="ExternalOutput")

    cum_w = np.zeros(cfg.NQ + 1, dtype=np.int64)
    np.cumsum(w_sched, out=cum_w[1:])
    NW = cfg.BATCH            # rhs free width per window (128)

    with TileContext(nc) as tc:
        with (
            tc.tile_pool(name="stp", bufs=globals().get("STP_BUFS", 6)) as stp,
            tc.tile_pool(name="psp", bufs=4, space="PSUM") as psp,
            tc.tile_pool(name="resp", bufs=8) as resp,
            tc.tile_pool(name="outp", bufs=4) as outp,
        ):
            RH = cfg.QG * cfg.B       # psum region height per quad (32)
            st_t = None
            ci = -1                   # current chunk index
            k0 = 0                    # first window of current chunk
            for bank in range(cfg.NBANKS):
                j0, j1 = bank * 16, min(bank * 16 + 16, cfg.NQ)
                res_t = resp.tile([128, 512], mybir.dt.float16)
                nc.gpsimd.dma_start(
                    out=res_t, in_=res[:, bank * 512 : (bank + 1) * 512])
                ps = psp.tile([128, 512], mybir.dt.float32)
                for j in range(j0, j1):
                    qr = j - j0
                    p0 = 32 * (qr % 4)
                    f0 = 128 * (qr // 4)
                    t_first = int(cum_w[j])
                    t_last = int(cum_w[j + 1]) - 1
                    for t in range(t_first, t_last + 1):
                        if ci + 1 < len(sizes) and t == int(cstarts[ci + 1]):
                            ci += 1
                            k0 = int(cstarts[ci])
                            n = sizes[ci]
                            st_t = stp.tile([128, n * PW], mybir.dt.float8e3)
                            nc.sync.dma_start(
                                out=st_t[:, :],
                                in_=st[:, k0 * PW : (k0 + n) * PW])
                        k = t - k0
                        nc.tensor.matmul(
                            ps[p0 : p0 + RH, f0 : f0 + 128],
                            st_t[:, k * PW + NW : (k + 1) * PW],
                            st_t[:, k * PW : k * PW + NW],
                            start=(t == t_first),
                            stop=(t == t_last),
                            tile_position=(0, p0),
                        )
                ot = outp.tile([128, 512], mybir.dt.float16)
                nc.vector.scalar_tensor_tensor(
                    ot, ps, 1.0 / WSCALE, res_t,
                    op0=mybir.AluOpType.mult, op1=mybir.AluOpType.add)
                out_eng = nc.sync if bank == cfg.NBANKS - 1 else nc.scalar
                out_eng.dma_start(out=out[:, bank * 512 : (bank + 1) * 512], in_=ot)
    if not nc.is_finalized():
        nc.finalize()
    return nc


def _decode(cfg, results, decode_quads):
    G, B, BATCH = cfg.G, cfg.B, cfg.BATCH
    outT = np.empty((G, B, BATCH), dtype=np.float32)
    for c in range(cfg.NCORES):
        res = np.asarray(results[c]["out"], dtype=np.float32)
        for j in range(cfg.NQ):
            bank, qr = j // 16, j % 16
            p0 = 32 * (qr % 4)
            f0 = bank * 512 + 128 * (qr // 4)
            blockv = res[p0 : p0 + cfg.QG * B, f0 : f0 + 128]
            genes = decode_quads[c][j]
            outT[genes] = blockv.reshape(cfg.QG, B, BATCH)
    return np.ascontiguousarray(outT.reshape(G * B, BATCH).T)


def _run(cfg, x, w, block_in, block_out, trace=False):
    in_maps, w_sched, decode_quads = _pack_host(cfg, x, w, block_in, block_out)
    nc = _build_nc(cfg, w_sched)
    r = run_bass_kernel_spmd(nc, in_maps, core_ids=list(range(cfg.NCORES)),
                             trace=trace)
    out = _decode(cfg, r.results, decode_quads)
    return out, r


def kernel(x, w, block_in, block_out):
    cfg = Cfg()
    out, _ = _run(cfg, x, w, block_in, block_out, trace=False)
    return out


# revision 18
# speedup vs baseline: 1.7195x; 1.0112x over previous
"""Trainium2 Bass kernel for nn_LinearPPI (block-sparse gene-gene message passing).

Computation (reference):
    out[b, 8*g_out + o] = sum_{n: block_out[n]=g_out} sum_i x[b, 8*block_in[n] + i] * w[n, i, o]
    out += x   (residual)

Strategy (v2, fp8 stream):
  - Blocks sorted by destination gene; destination genes sharded over 8 cores
    (edge/expert parallel, no collectives needed).
  - Per core, genes are packed into "quads" of QG=4 genes.  A quad owns a
    [32, 128] region of a PSUM bank (4 genes x 8 outs, 128 batch), laid out
    transposed (out^T).  16 quads fill one PSUM bank [128, 512] completely
    (M=32 matches the PE quadrant granularity), so the whole per-core output
    (125 quads) lives in 8 banks with no wasted partitions.
  - Work is a stream of "windows": 16 x-slabs (one slab = 8 rows of x^T for
    one source gene = [8, 128]) stacked to a [128, 128] rhs, and a matching
    scattered weight tile [128, 32] as lhsT.  One matmul per window:
        psum[p0:p0+32, f0:f0+128] (+)= lhsT.T @ rhs   (K=128, M=32, N=128)
  - Both x and w stream in float8 E3M4 (4 mantissa bits).  Weights are
    pre-scaled by 32 on the host so they sit in the e3m4 normal range; the
    1/32 descale is fused into the per-bank combine.  Measured end-to-end
    relative error ~1.2e-2 vs the 2e-2 gate.
  - The residual is NOT in the stream: per bank a [128, 512] fp16 tile with
    the bank's own-gene x^T values is DMA'd in, and a single DVE
    scalar_tensor_tensor computes  out_sbuf = psum * (1/32) + residual
    (bank-wide, fp16 out), which the Pool engine DMAs to HBM.
  - The x-slab gather is done on the host (indices are known at trace time),
    producing a sequential HBM stream -> all device DMAs are large and
    contiguous (memory-bound regime; model-exact DMA floor ~68us/core).
  - The per-core window schedule is made identical across cores (rank-sorted
    window-count maxima + zero-padding) so a single SPMD program serves all
    8 cores; per-core variation lives only in the streamed data.
  - Output is slot-ordered out^T; the host inverse-permutes, transposes and
    concatenates shards.  No all-reduce: destination sharding makes each
    core's output disjoint.
"""

import math
import numpy as np
import ml_dtypes

import concourse.bass as bass
import concourse.bacc as bacc
import concourse.mybir as mybir
from concourse.tile import TileContext
from concourse.bass_utils import run_bass_kernel_spmd

F8 = ml_dtypes.float8_e3m4
WSCALE = 32.0


class Cfg:
    def __init__(self, G=4000, B=8, BATCH=128, NCORES=8, chunk=24, qg=4):
        assert G % NCORES == 0
        self.G, self.B, self.BATCH, self.NCORES = G, B, BATCH, NCORES
        self.GPC = G // NCORES            # genes per core
        self.QG = qg                      # genes per quad (M = QG*B = 32)
        assert self.GPC % self.QG == 0
        self.NQ = self.GPC // self.QG     # quads per core (125)
        self.NBANKS = math.ceil(self.NQ / 16)
        self.SLOTS = 16                   # slabs per window (K = 128)
        self.CH = chunk                   # windows per DMA chunk
        self.TAIL_CH = 2                  # chunk size for the last CH windows
        self.PW = BATCH + self.QG * B     # stream bytes/row/window (160)

    def chunk_plan(self, w_tot):
        """Chunk sizes: full CH chunks, then TAIL_CH-sized tail chunks so the
        final bank's compute tail after the last DMA is short."""
        sizes = []
        rem = w_tot
        while rem > self.CH:
            sizes.append(self.CH)
            rem -= self.CH
        while rem > 0:
            take = min(self.TAIL_CH, rem)
            sizes.append(take)
            rem -= take
        starts = np.zeros(len(sizes) + 1, dtype=np.int64)
        np.cumsum(sizes, out=starts[1:])
        return list(sizes), starts


def _pack_host(cfg, x, w, block_in, block_out):
    """Sort/shard/pad on the host. Returns (in_maps, w_sched, decode_quads)."""
    G, B, BATCH, NC = cfg.G, cfg.B, cfg.BATCH, cfg.NCORES

    src = np.asarray(block_in, dtype=np.int64)
    dst = np.asarray(block_out, dtype=np.int64)
    w_s8 = None  # filled below after sorting

    order = np.argsort(dst, kind="stable")
    src_s = src[order]
    w_s8 = np.ascontiguousarray(np.asarray(w, dtype=np.float32)[order] * WSCALE
                                ).astype(F8)
    counts = np.bincount(dst, minlength=G)
    starts = np.zeros(G + 1, dtype=np.int64)
    np.cumsum(counts, out=starts[1:])

    # x^T slabs: xslab[g] = x[:, 8g:8g+8].T  -> [G, 8, BATCH]
    xslabf = np.ascontiguousarray(np.asarray(x, dtype=np.float32).T
                                  .reshape(G, B, BATCH))
    xslab8 = xslabf.astype(F8)
    xslab16 = xslabf.astype(np.float16)

    # --- balanced gene->core assignment (snake over count-sorted genes) ---
    order_g = np.argsort(-counts, kind="stable")
    core_of = np.empty(G, dtype=np.int64)
    for r in range(0, G, 2 * NC):
        blk = order_g[r : r + 2 * NC]
        pat = list(range(NC)) + list(range(NC - 1, -1, -1))
        for i, g in enumerate(blk):
            core_of[g] = pat[i]

    # --- per-core quad packing: target sums that are multiples of SLOTS ---
    per_core = []
    for c in range(NC):
        genes = np.where(core_of == c)[0]  # this core's genes
        pool = sorted(genes.tolist(), key=lambda g: -counts[g])
        quads = []
        for _ in range(cfg.NQ):
            q = [pool.pop(0)]                       # largest remaining
            while pool and len(q) < cfg.QG - 1:     # middle picks: big/small mix
                q.append(pool.pop(0) if len(q) % 2 else pool.pop(-1))
            if pool and len(q) < cfg.QG:
                s3 = sum(int(counts[g]) for g in q)
                # last pick: minimize padding to the next multiple of SLOTS
                best_i = min(range(len(pool)),
                             key=lambda i: (-(s3 + int(counts[pool[i]])))
                             % cfg.SLOTS)
                q.append(pool.pop(best_i))
            quads.append(q)
        assert not pool
        q_slabs = np.array([sum(int(counts[g]) for g in q) for q in quads])
        q_wins = np.ceil(q_slabs / cfg.SLOTS).astype(np.int64)
        q_wins = np.maximum(q_wins, 1)
        rank = np.argsort(-q_wins, kind="stable")
        per_core.append(([quads[j] for j in rank], q_wins[rank]))

    # common schedule: per rank, max window count over cores
    w_sched = np.max(np.stack([pc[1] for pc in per_core]), axis=0)
    cum_w = np.zeros(cfg.NQ + 1, dtype=np.int64)
    np.cumsum(w_sched, out=cum_w[1:])
    w_tot = int(cum_w[-1])

    # --- build per-core streams -------------------------------------------
    in_maps = []
    decode_quads = []
    for c in range(NC):
        quads_r, _ = per_core[c]
        slab_gene = np.full(w_tot * cfg.SLOTS, -1, dtype=np.int64)
        blk_ids, blk_pos, blk_rel = [], [], []
        for j in range(cfg.NQ):
            base = cum_w[j] * cfg.SLOTS
            p = 0
            for r, g in enumerate(quads_r[j]):
                s0, n = int(starts[g]), int(counts[g])
                ids = np.arange(s0, s0 + n)
                blk_ids.append(ids)
                blk_pos.append(base + p + np.arange(n))
                blk_rel.append(np.full(n, r, dtype=np.int64))
                p += n
            assert p <= int(w_sched[j]) * cfg.SLOTS
        blk_ids = np.concatenate(blk_ids)
        blk_pos = np.concatenate(blk_pos)
        blk_rel = np.concatenate(blk_rel)
        slab_gene[blk_pos] = src_s[blk_ids]

        # x slabs: [W, 128, BATCH] fp8
        xg = np.zeros((w_tot * cfg.SLOTS, B, BATCH), dtype=F8)
        m = slab_gene >= 0
        xg[m] = xslab8[slab_gene[m]]
        xg = xg.reshape(w_tot, cfg.SLOTS * B, BATCH)

        # scattered (pre-scaled) weights: [W, 128, 32] fp8
        wg5 = np.zeros((w_tot, cfg.SLOTS, B, cfg.QG, B), dtype=F8)
        wg5[blk_pos // cfg.SLOTS, blk_pos % cfg.SLOTS, :, blk_rel, :] = w_s8[blk_ids]
        wg = wg5.reshape(w_tot, cfg.SLOTS * B, cfg.QG * B)

        # combined stream, chunk-major along columns: chunk c of n windows is
        # a contiguous [128, n*PW] DRAM column block -> every DMA is a large
        # linear read (~650KB for full chunks).
        st = np.concatenate([xg, wg], axis=2)          # [W, 128, PW]
        sizes, cstarts = cfg.chunk_plan(w_tot)
        blocks = [
            st[cstarts[ci] : cstarts[ci] + n]
            .transpose(1, 0, 2).reshape(cfg.SLOTS * B, n * cfg.PW)
            for ci, n in enumerate(sizes)
        ]
        st = np.ascontiguousarray(np.concatenate(blocks, axis=1))

        # residual tiles: [128, NBANKS*512] fp16, laid out like the PSUM banks
        res = np.zeros((128, cfg.NBANKS * 512), dtype=np.float16)
        for j in range(cfg.NQ):
            bank, qr = j // 16, j % 16
            p0 = 32 * (qr % 4)
            f0 = bank * 512 + 128 * (qr // 4)
            genes = quads_r[j]
            res[p0 : p0 + cfg.QG * B, f0 : f0 + 128] = (
                xslab16[genes].reshape(cfg.QG * B, BATCH))

        in_maps.append({"st": st, "res": res})
        decode_quads.append(quads_r)

    return in_maps, w_sched, decode_quads


def _build_nc(cfg, w_sched):
    """Trace the (core-uniform) Bass program."""
    w_tot = int(np.sum(w_sched))
    PW = cfg.PW
    sizes, cstarts = cfg.chunk_plan(w_tot)
    nc = bacc.Bacc("TRN2")
    st = nc.dram_tensor("st", [128, w_tot * PW], mybir.dt.float8e3,
                        kind="ExternalInput")
    res = nc.dram_tensor("res", [128, cfg.NBANKS * 512], mybir.dt.float16,
                         kind="ExternalInput")
    out = nc.dram_tensor("out", [128, cfg.NBANKS * 512], mybir.dt.float16,
                         kind="ExternalOutput")

    cum_w = np.zeros(cfg.NQ + 1, dtype=np.int64)
    np.cumsum(w_sched, out=cum_w[1:])
    NW = cfg.BATCH            # rhs free width per window (128)

    with TileContext(nc) as tc:
        with (
            tc.tile_pool(name="stp", bufs=6) as stp,
            tc.tile_pool(name="psp", bufs=4, space="PSUM") as psp,
            tc.tile_pool(name="resp", bufs=8) as resp,
            tc.tile_pool(name="outp", bufs=4) as outp,
        ):
            RH = cfg.QG * cfg.B       # psum region height per quad (32)
            NHB = -(-cfg.NQ // 8)     # half-bank units of up to 8 quads
            st_t = None
            ci = -1                   # current chunk index
            k0 = 0                    # first window of current chunk
            for hb in range(NHB):
                j0, j1 = hb * 8, min(hb * 8 + 8, cfg.NQ)
                res_t = resp.tile([128, 256], mybir.dt.float16)
                nc.gpsimd.dma_start(
                    out=res_t, in_=res[:, hb * 256 : (hb + 1) * 256])
                ps = psp.tile([128, 256], mybir.dt.float32)
                for j in range(j0, j1):
                    qr = j - j0
                    p0 = 32 * (qr % 4)
                    f0 = 128 * (qr // 4)
                    t_first = int(cum_w[j])
                    t_last = int(cum_w[j + 1]) - 1
                    for t in range(t_first, t_last + 1):
                        if ci + 1 < len(sizes) and t == int(cstarts[ci + 1]):
                            ci += 1
                            k0 = int(cstarts[ci])
                            n = sizes[ci]
                            st_t = stp.tile([128, n * PW], mybir.dt.float8e3)
                            nc.sync.dma_start(
                                out=st_t[:, :],
                                in_=st[:, k0 * PW : (k0 + n) * PW])
                        k = t - k0
                        nc.tensor.matmul(
                            ps[p0 : p0 + RH, f0 : f0 + 128],
                            st_t[:, k * PW + NW : (k + 1) * PW],
                            st_t[:, k * PW : k * PW + NW],
                            start=(t == t_first),
                            stop=(t == t_last),
                            tile_position=(0, p0),
                        )
                ot = outp.tile([128, 256], mybir.dt.float16)
                nc.vector.scalar_tensor_tensor(
                    ot, ps, 1.0 / WSCALE, res_t,
                    op0=mybir.AluOpType.mult, op1=mybir.AluOpType.add)
                out_eng = nc.sync if hb == NHB - 1 else nc.scalar
                out_eng.dma_start(out=out[:, hb * 256 : (hb + 1) * 256], in_=ot)
    if not nc.is_finalized():
        nc.finalize()
    return nc


def _decode(cfg, results, decode_quads):
    G, B, BATCH = cfg.G, cfg.B, cfg.BATCH
    outT = np.empty((G, B, BATCH), dtype=np.float32)
    for c in range(cfg.NCORES):
        res = np.asarray(results[c]["out"], dtype=np.float32)
        for j in range(cfg.NQ):
            bank, qr = j // 16, j % 16
            p0 = 32 * (qr % 4)
            f0 = bank * 512 + 128 * (qr // 4)
            blockv = res[p0 : p0 + cfg.QG * B, f0 : f0 + 128]
            genes = decode_quads[c][j]
            outT[genes] = blockv.reshape(cfg.QG, B, BATCH)
    return np.ascontiguousarray(outT.reshape(G * B, BATCH).T)


def _run(cfg, x, w, block_in, block_out, trace=False):
    in_maps, w_sched, decode_quads = _pack_host(cfg, x, w, block_in, block_out)
    nc = _build_nc(cfg, w_sched)
    r = run_bass_kernel_spmd(nc, in_maps, core_ids=list(range(cfg.NCORES)),
                             trace=trace)
    out = _decode(cfg, r.results, decode_quads)
    return out, r


def kernel(x, w, block_in, block_out):
    cfg = Cfg()
    out, _ = _run(cfg, x, w, block_in, block_out, trace=False)
    return out


# revision 21
# speedup vs baseline: 1.8810x; 1.0939x over previous
"""Trainium2 Bass kernel for nn_LinearPPI (block-sparse gene-gene message passing).

Computation (reference):
    out[b, 8*g_out + o] = sum_{n: block_out[n]=g_out} sum_i x[b, 8*block_in[n] + i] * w[n, i, o]
    out += x   (residual)

Strategy (v3, fp8 stream, batch-major PSUM):
  - Blocks sorted by destination gene; destination genes sharded over 8 cores
    (edge/expert parallel, no collectives needed).
  - Per core, genes are packed into PAIRS (QG=2).  Work is a stream of
    "windows": 16 x-slabs (one slab = 8 rows of x^T for one source gene =
    [8, 128]) stacked to a [128, 128] tile, plus a scattered weight tile
    [128, 16] (16 slabs x 8x8 block at the slab's gene-of-pair column).
  - The matmul is BATCH-MAJOR: the x window is the STATIONARY operand
    (lhsT, [K=128, M=128 batch]) and the weight tile is the MOVING operand
    (rhs, [K=128, N=16]).  One matmul per window:
        psum[0:128, c0:c0+16] (+)= x_win.T @ w_win   (K=128, M=128, N=16)
    Pair output regions are free-dim column ranges, so there is no PE
    32-partition quadrant constraint: QG=2 halves the zero-padding of the
    scattered weight tile vs QG=4 (50% vs 25% density), and PSUM banks pack
    densely (32 pairs x 16 cols = one [128, 512] bank; 250 pairs < 8 banks).
  - Both x and w stream in float8 E3M4 (4 mantissa bits).  Weights are
    pre-scaled by 32 on the host so they sit in the e3m4 normal range; the
    1/32 descale is fused into the combine.  Measured end-to-end relative
    error ~1.2e-2 vs the 2e-2 gate.
  - The residual is NOT in the stream: per half-bank (16 pairs = [128, 256])
    an fp16 tile holding the pairs' own-gene x columns (batch-major, so it is
    a direct column gather of x) is DMA'd in, and a single DVE
    scalar_tensor_tensor computes  out_sbuf = psum * (1/32) + residual,
    which is DMA'd to HBM as fp16.
  - The x-slab gather is done on the host (indices are known at trace time),
    producing a sequential HBM stream -> all device DMAs are large and
    contiguous (memory-bound regime; model DMA floor ~60us/core).
  - The per-core window schedule is made identical across cores (rank-sorted
    window-count maxima + zero-padding) so a single SPMD program serves all
    8 cores; per-core variation lives only in the streamed data.
  - Output is slot-ordered batch-major; the host inverse-permutes columns and
    concatenates shards.  No all-reduce: destination sharding makes each
    core's output disjoint.
"""

import math
import numpy as np
import ml_dtypes

import concourse.bacc as bacc
import concourse.mybir as mybir
from concourse.tile import TileContext
from concourse.bass_utils import run_bass_kernel_spmd

F8 = ml_dtypes.float8_e3m4
WSCALE = 32.0


class Cfg:
    def __init__(self, G=4000, B=8, BATCH=128, NCORES=8, chunk=32, qg=2):
        assert G % NCORES == 0
        self.G, self.B, self.BATCH, self.NCORES = G, B, BATCH, NCORES
        self.GPC = G // NCORES            # genes per core
        self.QG = qg                      # genes per pair
        assert self.GPC % self.QG == 0
        self.NQ = self.GPC // self.QG     # pairs per core (250)
        self.QW = self.QG * B             # psum cols per pair (16)
        self.NHB = math.ceil(self.NQ / 16)  # half-bank units of 16 pairs
        self.SLOTS = 16                   # slabs per window (K = 128)
        self.CH = chunk                   # windows per DMA chunk
        self.TAIL_CH = 4                  # chunk size for the last CH windows
        self.PW = BATCH + self.QW         # stream bytes/row/window (144)

    def chunk_plan(self, w_tot):
        """Chunk sizes: full CH chunks, then TAIL_CH-sized tail chunks so the
        final half-bank's compute tail after the last DMA is short."""
        sizes = []
        rem = w_tot
        while rem > self.CH:
            sizes.append(self.CH)
            rem -= self.CH
        while rem > 0:
            take = min(self.TAIL_CH, rem)
            sizes.append(take)
            rem -= take
        starts = np.zeros(len(sizes) + 1, dtype=np.int64)
        np.cumsum(sizes, out=starts[1:])
        return list(sizes), starts


def _pack_host(cfg, x, w, block_in, block_out):
    """Sort/shard/pad on the host. Returns (in_maps, w_sched, decode_quads)."""
    G, B, BATCH, NC = cfg.G, cfg.B, cfg.BATCH, cfg.NCORES

    src = np.asarray(block_in, dtype=np.int64)
    dst = np.asarray(block_out, dtype=np.int64)

    order = np.argsort(dst, kind="stable")
    src_s = src[order]
    w_s8 = np.ascontiguousarray(np.asarray(w, dtype=np.float32)[order] * WSCALE
                                ).astype(F8)
    counts = np.bincount(dst, minlength=G)
    starts = np.zeros(G + 1, dtype=np.int64)
    np.cumsum(counts, out=starts[1:])

    xf = np.asarray(x, dtype=np.float32)
    # x^T slabs: xslab[g] = x[:, 8g:8g+8].T  -> [G, 8, BATCH], fp8
    xslab8 = np.ascontiguousarray(xf.T.reshape(G, B, BATCH)).astype(F8)
    x16 = xf.astype(np.float16)            # batch-major residual source

    # --- balanced gene->core assignment (snake over count-sorted genes) ---
    order_g = np.argsort(-counts, kind="stable")
    core_of = np.empty(G, dtype=np.int64)
    for r in range(0, G, 2 * NC):
        blk = order_g[r : r + 2 * NC]
        pat = list(range(NC)) + list(range(NC - 1, -1, -1))
        for i, g in enumerate(blk):
            core_of[g] = pat[i]

    # --- per-core pair packing: target sums that are multiples of SLOTS ---
    per_core = []
    for c in range(NC):
        genes = np.where(core_of == c)[0]  # this core's genes
        pool = sorted(genes.tolist(), key=lambda g: -counts[g])
        quads = []
        for _ in range(cfg.NQ):
            q = [pool.pop(0)]                       # largest remaining
            while pool and len(q) < cfg.QG - 1:     # middle picks: big/small mix
                q.append(pool.pop(0) if len(q) % 2 else pool.pop(-1))
            if pool and len(q) < cfg.QG:
                s3 = sum(int(counts[g]) for g in q)
                # last pick: minimize padding to the next multiple of SLOTS
                best_i = min(range(len(pool)),
                             key=lambda i: (-(s3 + int(counts[pool[i]])))
                             % cfg.SLOTS)
                q.append(pool.pop(best_i))
            quads.append(q)
        assert not pool
        q_slabs = np.array([sum(int(counts[g]) for g in q) for q in quads])
        q_wins = np.ceil(q_slabs / cfg.SLOTS).astype(np.int64)
        q_wins = np.maximum(q_wins, 1)
        rank = np.argsort(-q_wins, kind="stable")
        per_core.append(([quads[j] for j in rank], q_wins[rank]))

    # common schedule: per rank, max window count over cores
    w_sched = np.max(np.stack([pc[1] for pc in per_core]), axis=0)
    cum_w = np.zeros(cfg.NQ + 1, dtype=np.int64)
    np.cumsum(w_sched, out=cum_w[1:])
    w_tot = int(cum_w[-1])

    # --- build per-core streams -------------------------------------------
    in_maps = []
    decode_quads = []
    for c in range(NC):
        quads_r, _ = per_core[c]
        slab_gene = np.full(w_tot * cfg.SLOTS, -1, dtype=np.int64)
        blk_ids, blk_pos, blk_rel = [], [], []
        for j in range(cfg.NQ):
            base = cum_w[j] * cfg.SLOTS
            p = 0
            for r, g in enumerate(quads_r[j]):
                s0, n = int(starts[g]), int(counts[g])
                ids = np.arange(s0, s0 + n)
                blk_ids.append(ids)
                blk_pos.append(base + p + np.arange(n))
                blk_rel.append(np.full(n, r, dtype=np.int64))
                p += n
            assert p <= int(w_sched[j]) * cfg.SLOTS
        blk_ids = np.concatenate(blk_ids)
        blk_pos = np.concatenate(blk_pos)
        blk_rel = np.concatenate(blk_rel)
        slab_gene[blk_pos] = src_s[blk_ids]

        # x slabs: [W, 128, BATCH] fp8
        xg = np.zeros((w_tot * cfg.SLOTS, B, BATCH), dtype=F8)
        m = slab_gene >= 0
        xg[m] = xslab8[slab_gene[m]]
        xg = xg.reshape(w_tot, cfg.SLOTS * B, BATCH)

        # scattered (pre-scaled) weights: [W, 128, 16] fp8
        wg5 = np.zeros((w_tot, cfg.SLOTS, B, cfg.QG, B), dtype=F8)
        wg5[blk_pos // cfg.SLOTS, blk_pos % cfg.SLOTS, :, blk_rel, :] = w_s8[blk_ids]
        wg = wg5.reshape(w_tot, cfg.SLOTS * B, cfg.QW)

        # combined stream, chunk-major along columns: chunk c of n windows is
        # a contiguous [128, n*PW] DRAM column block -> every DMA is a large
        # linear read (~440KB for full chunks).
        st = np.concatenate([xg, wg], axis=2)          # [W, 128, PW]
        sizes, cstarts = cfg.chunk_plan(w_tot)
        blocks = [
            st[cstarts[ci] : cstarts[ci] + n]
            .transpose(1, 0, 2).reshape(cfg.SLOTS * B, n * cfg.PW)
            for ci, n in enumerate(sizes)
        ]
        st = np.ascontiguousarray(np.concatenate(blocks, axis=1))

        # residual tiles: batch-major [128, NHB*256] fp16; pair j's genes at
        # cols hb*256 + slot*16 + r*8 (mirrors the PSUM column layout)
        res = np.zeros((128, cfg.NHB * 256), dtype=np.float16)
        for j in range(cfg.NQ):
            hb, slot = j // 16, j % 16
            for r, g in enumerate(quads_r[j]):
                col = hb * 256 + slot * 16 + r * B
                res[:, col : col + B] = x16[:, g * B : (g + 1) * B]

        in_maps.append({"st": st, "res": res})
        decode_quads.append(quads_r)

    return in_maps, w_sched, decode_quads


def _build_nc(cfg, w_sched):
    """Trace the (core-uniform) Bass program."""
    w_tot = int(np.sum(w_sched))
    PW = cfg.PW
    sizes, cstarts = cfg.chunk_plan(w_tot)
    nc = bacc.Bacc("TRN2")
    st = nc.dram_tensor("st", [128, w_tot * PW], mybir.dt.float8e3,
                        kind="ExternalInput")
    res = nc.dram_tensor("res", [128, cfg.NHB * 256], mybir.dt.float16,
                         kind="ExternalInput")
    out = nc.dram_tensor("out", [128, cfg.NHB * 256], mybir.dt.float16,
                         kind="ExternalOutput")

    cum_w = np.zeros(cfg.NQ + 1, dtype=np.int64)
    np.cumsum(w_sched, out=cum_w[1:])
    NW = cfg.BATCH            # x section width per window (128)

    with TileContext(nc) as tc:
        with (
            tc.tile_pool(name="stp", bufs=6) as stp,
            tc.tile_pool(name="psp", bufs=4, space="PSUM") as psp,
            tc.tile_pool(name="resp", bufs=8) as resp,
            tc.tile_pool(name="outp", bufs=4) as outp,
        ):
            st_t = None
            ci = -1                   # current chunk index
            k0 = 0                    # first window of current chunk
            for hb in range(cfg.NHB):
                j0, j1 = hb * 16, min(hb * 16 + 16, cfg.NQ)
                res_t = resp.tile([128, 256], mybir.dt.float16)
                nc.gpsimd.dma_start(
                    out=res_t, in_=res[:, hb * 256 : (hb + 1) * 256])
                ps = psp.tile([128, 256], mybir.dt.float32)
                for j in range(j0, j1):
                    c0 = cfg.QW * (j - j0)
                    t_first = int(cum_w[j])
                    t_last = int(cum_w[j + 1]) - 1
                    for t in range(t_first, t_last + 1):
                        if ci + 1 < len(sizes) and t == int(cstarts[ci + 1]):
                            ci += 1
                            k0 = int(cstarts[ci])
                            n = sizes[ci]
                            st_t = stp.tile([128, n * PW], mybir.dt.float8e3)
                            nc.sync.dma_start(
                                out=st_t[:, :],
                                in_=st[:, k0 * PW : (k0 + n) * PW])
                        k = t - k0
                        nc.tensor.matmul(
                            ps[:, c0 : c0 + cfg.QW],
                            st_t[:, k * PW : k * PW + NW],
                            st_t[:, k * PW + NW : (k + 1) * PW],
                            start=(t == t_first),
                            stop=(t == t_last),
                        )
                ot = outp.tile([128, 256], mybir.dt.float16)
                nc.vector.scalar_tensor_tensor(
                    ot, ps, 1.0 / WSCALE, res_t,
                    op0=mybir.AluOpType.mult, op1=mybir.AluOpType.add)
                out_eng = nc.sync if hb == cfg.NHB - 1 else nc.scalar
                out_eng.dma_start(out=out[:, hb * 256 : (hb + 1) * 256], in_=ot)
    if not nc.is_finalized():
        nc.finalize()
    return nc


def _decode(cfg, results, decode_quads):
    G, B, BATCH = cfg.G, cfg.B, cfg.BATCH
    full = np.empty((BATCH, G * B), dtype=np.float32)
    for c in range(cfg.NCORES):
        res = np.asarray(results[c]["out"], dtype=np.float32)
        for j in range(cfg.NQ):
            hb, slot = j // 16, j % 16
            for r, g in enumerate(decode_quads[c][j]):
                col = hb * 256 + slot * 16 + r * B
                full[:, g * B : (g + 1) * B] = res[:, col : col + B]
    return full


def _run(cfg, x, w, block_in, block_out, trace=False):
    in_maps, w_sched, decode_quads = _pack_host(cfg, x, w, block_in, block_out)
    nc = _build_nc(cfg, w_sched)
    r = run_bass_kernel_spmd(nc, in_maps, core_ids=list(range(cfg.NCORES)),
                             trace=trace)
    out = _decode(cfg, r.results, decode_quads)
    return out, r


def kernel(x, w, block_in, block_out):
    cfg = Cfg()
    out, _ = _run(cfg, x, w, block_in, block_out, trace=False)
    return out


# revision 22
# speedup vs baseline: 1.9027x; 1.0115x over previous
"""Trainium2 Bass kernel for nn_LinearPPI (block-sparse gene-gene message passing).

Computation (reference):
    out[b, 8*g_out + o] = sum_{n: block_out[n]=g_out} sum_i x[b, 8*block_in[n] + i] * w[n, i, o]
    out += x   (residual)

Strategy (v3, fp8 stream, batch-major PSUM):
  - Blocks sorted by destination gene; destination genes sharded over 8 cores
    (edge/expert parallel, no collectives needed).
  - Per core, genes are packed into PAIRS (QG=2).  Work is a stream of
    "windows": 16 x-slabs (one slab = 8 rows of x^T for one source gene =
    [8, 128]) stacked to a [128, 128] tile, plus a scattered weight tile
    [128, 16] (16 slabs x 8x8 block at the slab's gene-of-pair column).
  - The matmul is BATCH-MAJOR: the x window is the STATIONARY operand
    (lhsT, [K=128, M=128 batch]) and the weight tile is the MOVING operand
    (rhs, [K=128, N=16]).  One matmul per window:
        psum[0:128, c0:c0+16] (+)= x_win.T @ w_win   (K=128, M=128, N=16)
    Pair output regions are free-dim column ranges, so there is no PE
    32-partition quadrant constraint: QG=2 halves the zero-padding of the
    scattered weight tile vs QG=4 (50% vs 25% density), and PSUM banks pack
    densely (32 pairs x 16 cols = one [128, 512] bank; 250 pairs < 8 banks).
  - Both x and w stream in float8 E3M4 (4 mantissa bits).  Weights are
    pre-scaled by 32 on the host so they sit in the e3m4 normal range; the
    1/32 descale is fused into the combine.  Measured end-to-end relative
    error ~1.2e-2 vs the 2e-2 gate.
  - The residual is NOT in the stream: per half-bank (16 pairs = [128, 256])
    an fp16 tile holding the pairs' own-gene x columns (batch-major, so it is
    a direct column gather of x) is DMA'd in, and a single DVE
    scalar_tensor_tensor computes  out_sbuf = psum * (1/32) + residual,
    which is DMA'd to HBM as fp16.
  - The x-slab gather is done on the host (indices are known at trace time),
    producing a sequential HBM stream -> all device DMAs are large and
    contiguous (memory-bound regime; model DMA floor ~59us/core, achieved
    ~65us/core vs ~124us for the fp16 QG=2-quadrant baseline).
  - The per-core window schedule is made identical across cores (rank-sorted
    window-count maxima + zero-padding) so a single SPMD program serves all
    8 cores; per-core variation lives only in the streamed data.
  - Output is slot-ordered batch-major; the host inverse-permutes columns and
    concatenates shards.  No all-reduce: destination sharding makes each
    core's output disjoint.
"""

import math
import numpy as np
import ml_dtypes

import concourse.bacc as bacc
import concourse.mybir as mybir
from concourse.tile import TileContext
from concourse.bass_utils import run_bass_kernel_spmd

F8 = ml_dtypes.float8_e3m4
WSCALE = 32.0


class Cfg:
    def __init__(self, G=4000, B=8, BATCH=128, NCORES=8, chunk=32, qg=2):
        assert G % NCORES == 0
        self.G, self.B, self.BATCH, self.NCORES = G, B, BATCH, NCORES
        self.GPC = G // NCORES            # genes per core
        self.QG = qg                      # genes per pair
        assert self.GPC % self.QG == 0
        self.NQ = self.GPC // self.QG     # pairs per core (250)
        self.QW = self.QG * B             # psum cols per pair (16)
        self.NHB = math.ceil(self.NQ / 16)  # half-bank units of 16 pairs
        self.SLOTS = 16                   # slabs per window (K = 128)
        self.CH = chunk                   # windows per DMA chunk
        self.TAIL_CH = 4                  # chunk size for the last CH windows
        self.PW = BATCH + self.QW         # stream bytes/row/window (144)

    def chunk_plan(self, w_tot):
        """Chunk sizes: full CH chunks, then TAIL_CH-sized tail chunks so the
        final half-bank's compute tail after the last DMA is short."""
        sizes = []
        rem = w_tot
        while rem > self.CH:
            sizes.append(self.CH)
            rem -= self.CH
        while rem > 0:
            take = min(self.TAIL_CH, rem)
            sizes.append(take)
            rem -= take
        starts = np.zeros(len(sizes) + 1, dtype=np.int64)
        np.cumsum(sizes, out=starts[1:])
        return list(sizes), starts


def _pack_host(cfg, x, w, block_in, block_out):
    """Sort/shard/pad on the host. Returns (in_maps, w_sched, decode_quads)."""
    G, B, BATCH, NC = cfg.G, cfg.B, cfg.BATCH, cfg.NCORES

    src = np.asarray(block_in, dtype=np.int64)
    dst = np.asarray(block_out, dtype=np.int64)

    order = np.argsort(dst, kind="stable")
    src_s = src[order]
    w_s8 = np.ascontiguousarray(np.asarray(w, dtype=np.float32)[order] * WSCALE
                                ).astype(F8)
    counts = np.bincount(dst, minlength=G)
    starts = np.zeros(G + 1, dtype=np.int64)
    np.cumsum(counts, out=starts[1:])

    xf = np.asarray(x, dtype=np.float32)
    # x^T slabs: xslab[g] = x[:, 8g:8g+8].T  -> [G, 8, BATCH], fp8
    xslab8 = np.ascontiguousarray(xf.T.reshape(G, B, BATCH)).astype(F8)
    x16 = xf.astype(np.float16)            # batch-major residual source

    # --- balanced gene->core assignment (snake over count-sorted genes) ---
    order_g = np.argsort(-counts, kind="stable")
    core_of = np.empty(G, dtype=np.int64)
    for r in range(0, G, 2 * NC):
        blk = order_g[r : r + 2 * NC]
        pat = list(range(NC)) + list(range(NC - 1, -1, -1))
        for i, g in enumerate(blk):
            core_of[g] = pat[i]

    # --- per-core pair packing: target sums that are multiples of SLOTS ---
    per_core = []
    for c in range(NC):
        genes = np.where(core_of == c)[0]  # this core's genes
        pool = sorted(genes.tolist(), key=lambda g: -counts[g])
        quads = []
        for _ in range(cfg.NQ):
            q = [pool.pop(0)]                       # largest remaining
            while pool and len(q) < cfg.QG - 1:     # middle picks: big/small mix
                q.append(pool.pop(0) if len(q) % 2 else pool.pop(-1))
            if pool and len(q) < cfg.QG:
                s3 = sum(int(counts[g]) for g in q)
                # last pick: minimize padding to the next multiple of SLOTS
                best_i = min(range(len(pool)),
                             key=lambda i: (-(s3 + int(counts[pool[i]])))
                             % cfg.SLOTS)
                q.append(pool.pop(best_i))
            quads.append(q)
        assert not pool
        q_slabs = np.array([sum(int(counts[g]) for g in q) for q in quads])
        q_wins = np.ceil(q_slabs / cfg.SLOTS).astype(np.int64)
        q_wins = np.maximum(q_wins, 1)
        rank = np.argsort(-q_wins, kind="stable")
        per_core.append(([quads[j] for j in rank], q_wins[rank]))

    # common schedule: per rank, max window count over cores
    w_sched = np.max(np.stack([pc[1] for pc in per_core]), axis=0)
    cum_w = np.zeros(cfg.NQ + 1, dtype=np.int64)
    np.cumsum(w_sched, out=cum_w[1:])
    w_tot = int(cum_w[-1])

    # --- build per-core streams -------------------------------------------
    in_maps = []
    decode_quads = []
    for c in range(NC):
        quads_r, _ = per_core[c]
        slab_gene = np.full(w_tot * cfg.SLOTS, -1, dtype=np.int64)
        blk_ids, blk_pos, blk_rel = [], [], []
        for j in range(cfg.NQ):
            base = cum_w[j] * cfg.SLOTS
            p = 0
            for r, g in enumerate(quads_r[j]):
                s0, n = int(starts[g]), int(counts[g])
                ids = np.arange(s0, s0 + n)
                blk_ids.append(ids)
                blk_pos.append(base + p + np.arange(n))
                blk_rel.append(np.full(n, r, dtype=np.int64))
                p += n
            assert p <= int(w_sched[j]) * cfg.SLOTS
        blk_ids = np.concatenate(blk_ids)
        blk_pos = np.concatenate(blk_pos)
        blk_rel = np.concatenate(blk_rel)
        slab_gene[blk_pos] = src_s[blk_ids]

        # x slabs: [W, 128, BATCH] fp8
        xg = np.zeros((w_tot * cfg.SLOTS, B, BATCH), dtype=F8)
        m = slab_gene >= 0
        xg[m] = xslab8[slab_gene[m]]
        xg = xg.reshape(w_tot, cfg.SLOTS * B, BATCH)

        # scattered (pre-scaled) weights: [W, 128, 16] fp8
        wg5 = np.zeros((w_tot, cfg.SLOTS, B, cfg.QG, B), dtype=F8)
        wg5[blk_pos // cfg.SLOTS, blk_pos % cfg.SLOTS, :, blk_rel, :] = w_s8[blk_ids]
        wg = wg5.reshape(w_tot, cfg.SLOTS * B, cfg.QW)

        # combined stream, chunk-major along columns: chunk c of n windows is
        # a contiguous [128, n*PW] DRAM column block -> every DMA is a large
        # linear read (~440KB for full chunks).
        st = np.concatenate([xg, wg], axis=2)          # [W, 128, PW]
        sizes, cstarts = cfg.chunk_plan(w_tot)
        blocks = [
            st[cstarts[ci] : cstarts[ci] + n]
            .transpose(1, 0, 2).reshape(cfg.SLOTS * B, n * cfg.PW)
            for ci, n in enumerate(sizes)
        ]
        st = np.ascontiguousarray(np.concatenate(blocks, axis=1))

        # residual tiles: batch-major [128, NHB*256] fp16; pair j's genes at
        # cols hb*256 + slot*16 + r*8 (mirrors the PSUM column layout)
        res = np.zeros((128, cfg.NHB * 256), dtype=np.float16)
        for j in range(cfg.NQ):
            hb, slot = j // 16, j % 16
            for r, g in enumerate(quads_r[j]):
                col = hb * 256 + slot * 16 + r * B
                res[:, col : col + B] = x16[:, g * B : (g + 1) * B]

        in_maps.append({"st": st, "res": res})
        decode_quads.append(quads_r)

    return in_maps, w_sched, decode_quads


def _build_nc(cfg, w_sched):
    """Trace the (core-uniform) Bass program."""
    w_tot = int(np.sum(w_sched))
    PW = cfg.PW
    sizes, cstarts = cfg.chunk_plan(w_tot)
    nc = bacc.Bacc("TRN2")
    st = nc.dram_tensor("st", [128, w_tot * PW], mybir.dt.float8e3,
                        kind="ExternalInput")
    res = nc.dram_tensor("res", [128, cfg.NHB * 256], mybir.dt.float16,
                         kind="ExternalInput")
    out = nc.dram_tensor("out", [128, cfg.NHB * 256], mybir.dt.float16,
                         kind="ExternalOutput")

    cum_w = np.zeros(cfg.NQ + 1, dtype=np.int64)
    np.cumsum(w_sched, out=cum_w[1:])
    NW = cfg.BATCH            # x section width per window (128)

    with TileContext(nc) as tc:
        with (
            tc.tile_pool(name="stp", bufs=6) as stp,
            tc.tile_pool(name="psp", bufs=4, space="PSUM") as psp,
            tc.tile_pool(name="resp", bufs=8) as resp,
            tc.tile_pool(name="outp", bufs=4) as outp,
        ):
            st_t = None
            ci = -1                   # current chunk index
            k0 = 0                    # first window of current chunk
            for hb in range(cfg.NHB):
                j0, j1 = hb * 16, min(hb * 16 + 16, cfg.NQ)
                res_t = resp.tile([128, 256], mybir.dt.float16)
                nc.gpsimd.dma_start(
                    out=res_t, in_=res[:, hb * 256 : (hb + 1) * 256])
                ps = psp.tile([128, 256], mybir.dt.float32)
                for j in range(j0, j1):
                    c0 = cfg.QW * (j - j0)
                    t_first = int(cum_w[j])
                    t_last = int(cum_w[j + 1]) - 1
                    for t in range(t_first, t_last + 1):
                        if ci + 1 < len(sizes) and t == int(cstarts[ci + 1]):
                            ci += 1
                            k0 = int(cstarts[ci])
                            n = sizes[ci]
                            st_t = stp.tile([128, n * PW], mybir.dt.float8e3)
                            nc.sync.dma_start(
                                out=st_t[:, :],
                                in_=st[:, k0 * PW : (k0 + n) * PW])
                        k = t - k0
                        nc.tensor.matmul(
                            ps[:, c0 : c0 + cfg.QW],
                            st_t[:, k * PW : k * PW + NW],
                            st_t[:, k * PW + NW : (k + 1) * PW],
                            start=(t == t_first),
                            stop=(t == t_last),
                        )
                ot = outp.tile([128, 256], mybir.dt.float16)
                nc.vector.scalar_tensor_tensor(
                    ot, ps, 1.0 / WSCALE, res_t,
                    op0=mybir.AluOpType.mult, op1=mybir.AluOpType.add)
                out_eng = nc.sync if hb == cfg.NHB - 1 else nc.scalar
                out_eng.dma_start(out=out[:, hb * 256 : (hb + 1) * 256], in_=ot)
    if not nc.is_finalized():
        nc.finalize()
    return nc


def _decode(cfg, results, decode_quads):
    G, B, BATCH = cfg.G, cfg.B, cfg.BATCH
    full = np.empty((BATCH, G * B), dtype=np.float32)
    for c in range(cfg.NCORES):
        res = np.asarray(results[c]["out"], dtype=np.float32)
        for j in range(cfg.NQ):
            hb, slot = j // 16, j % 16
            for r, g in enumerate(decode_quads[c][j]):
                col = hb * 256 + slot * 16 + r * B
                full[:, g * B : (g + 1) * B] = res[:, col : col + B]
    return full


def _run(cfg, x, w, block_in, block_out, trace=False):
    in_maps, w_sched, decode_quads = _pack_host(cfg, x, w, block_in, block_out)
    nc = _build_nc(cfg, w_sched)
    r = run_bass_kernel_spmd(nc, in_maps, core_ids=list(range(cfg.NCORES)),
                             trace=trace)
    out = _decode(cfg, r.results, decode_quads)
    return out, r


def kernel(x, w, block_in, block_out):
    cfg = Cfg()
    out, _ = _run(cfg, x, w, block_in, block_out, trace=False)
    return out


# revision 23
# speedup vs baseline: 1.9394x; 1.0193x over previous
"""Trainium2 Bass kernel for nn_LinearPPI (block-sparse gene-gene message passing).

Computation (reference):
    out[b, 8*g_out + o] = sum_{n: block_out[n]=g_out} sum_i x[b, 8*block_in[n] + i] * w[n, i, o]
    out += x   (residual)

Strategy (v3, fp8 stream, batch-major PSUM):
  - Blocks sorted by destination gene; destination genes sharded over 8 cores
    (edge/expert parallel, no collectives needed).
  - Per core, genes are packed into PAIRS (QG=2).  Work is a stream of
    "windows": 16 x-slabs (one slab = 8 rows of x^T for one source gene =
    [8, 128]) stacked to a [128, 128] tile, plus a scattered weight tile
    [128, 16] (16 slabs x 8x8 block at the slab's gene-of-pair column).
  - The matmul is BATCH-MAJOR: the x window is the STATIONARY operand
    (lhsT, [K=128, M=128 batch]) and the weight tile is the MOVING operand
    (rhs, [K=128, N=16]).  One matmul per window:
        psum[0:128, c0:c0+16] (+)= x_win.T @ w_win   (K=128, M=128, N=16)
    Pair output regions are free-dim column ranges, so there is no PE
    32-partition quadrant constraint: QG=2 halves the zero-padding of the
    scattered weight tile vs QG=4 (50% vs 25% density), and PSUM banks pack
    densely (32 pairs x 16 cols = one [128, 512] bank; 250 pairs < 8 banks).
  - Both x and w stream in float8 E3M4 (4 mantissa bits).  Weights are
    pre-scaled by 32 on the host so they sit in the e3m4 normal range; the
    1/32 descale is fused into the combine.  Measured end-to-end relative
    error ~1.2e-2 vs the 2e-2 gate.
  - The residual is NOT in the stream: per half-bank (16 pairs = [128, 256])
    an fp16 tile holding the pairs' own-gene x columns (batch-major, so it is
    a direct column gather of x) is DMA'd in, and a single DVE
    scalar_tensor_tensor computes  out_sbuf = psum * (1/32) + residual,
    which is DMA'd to HBM as fp16.
  - The x-slab gather is done on the host (indices are known at trace time),
    producing a sequential HBM stream -> all device DMAs are large and
    contiguous (memory-bound regime; model DMA floor ~59us/core, achieved
    ~65us/core vs ~124us for the fp16 QG=2-quadrant baseline).
  - The per-core window schedule is made identical across cores (rank-sorted
    window-count maxima + zero-padding) so a single SPMD program serves all
    8 cores; per-core variation lives only in the streamed data.
  - Output is slot-ordered batch-major; the host inverse-permutes columns and
    concatenates shards.  No all-reduce: destination sharding makes each
    core's output disjoint.
"""

import math
import numpy as np
import ml_dtypes

import concourse.bacc as bacc
import concourse.mybir as mybir
from concourse.tile import TileContext
from concourse.bass_utils import run_bass_kernel_spmd

F8 = ml_dtypes.float8_e3m4
WSCALE = 32.0


class Cfg:
    def __init__(self, G=4000, B=8, BATCH=128, NCORES=8, chunk=32, qg=2):
        assert G % NCORES == 0
        self.G, self.B, self.BATCH, self.NCORES = G, B, BATCH, NCORES
        self.GPC = G // NCORES            # genes per core
        self.QG = qg                      # genes per pair
        assert self.GPC % self.QG == 0
        self.NQ = self.GPC // self.QG     # pairs per core (250)
        self.QW = self.QG * B             # psum cols per pair (16)
        self.NHB = math.ceil(self.NQ / 16)  # half-bank units of 16 pairs
        self.SLOTS = 16                   # slabs per window (K = 128)
        self.CH = chunk                   # windows per DMA chunk
        self.TAIL_CH = 4                  # chunk size for the last CH windows
        self.PW = BATCH + self.QW         # stream bytes/row/window (144)

    def chunk_plan(self, w_tot):
        """Chunk sizes: full CH chunks, then TAIL_CH-sized tail chunks so the
        final half-bank's compute tail after the last DMA is short."""
        sizes = []
        rem = w_tot
        while rem > self.CH:
            sizes.append(self.CH)
            rem -= self.CH
        while rem > 0:
            take = min(self.TAIL_CH, rem)
            sizes.append(take)
            rem -= take
        starts = np.zeros(len(sizes) + 1, dtype=np.int64)
        np.cumsum(sizes, out=starts[1:])
        return list(sizes), starts


def _pack_host(cfg, x, w, block_in, block_out):
    """Sort/shard/pad on the host. Returns (in_maps, w_sched, decode_quads)."""
    G, B, BATCH, NC = cfg.G, cfg.B, cfg.BATCH, cfg.NCORES

    src = np.asarray(block_in, dtype=np.int64)
    dst = np.asarray(block_out, dtype=np.int64)

    order = np.argsort(dst, kind="stable")
    src_s = src[order]
    w_s8 = np.ascontiguousarray(np.asarray(w, dtype=np.float32)[order] * WSCALE
                                ).astype(F8)
    counts = np.bincount(dst, minlength=G)
    starts = np.zeros(G + 1, dtype=np.int64)
    np.cumsum(counts, out=starts[1:])

    xf = np.asarray(x, dtype=np.float32)
    # x^T slabs: xslab[g] = x[:, 8g:8g+8].T  -> [G, 8, BATCH], fp8
    xslab8 = np.ascontiguousarray(xf.T.reshape(G, B, BATCH)).astype(F8)
    x8r = xf.astype(F8)                    # batch-major residual source

    # --- balanced gene->core assignment (snake over count-sorted genes) ---
    order_g = np.argsort(-counts, kind="stable")
    core_of = np.empty(G, dtype=np.int64)
    for r in range(0, G, 2 * NC):
        blk = order_g[r : r + 2 * NC]
        pat = list(range(NC)) + list(range(NC - 1, -1, -1))
        for i, g in enumerate(blk):
            core_of[g] = pat[i]

    # --- per-core pair packing: target sums that are multiples of SLOTS ---
    per_core = []
    for c in range(NC):
        genes = np.where(core_of == c)[0]  # this core's genes
        pool = sorted(genes.tolist(), key=lambda g: -counts[g])
        quads = []
        for _ in range(cfg.NQ):
            q = [pool.pop(0)]                       # largest remaining
            while pool and len(q) < cfg.QG - 1:     # middle picks: big/small mix
                q.append(pool.pop(0) if len(q) % 2 else pool.pop(-1))
            if pool and len(q) < cfg.QG:
                s3 = sum(int(counts[g]) for g in q)
                # last pick: minimize padding to the next multiple of SLOTS
                best_i = min(range(len(pool)),
                             key=lambda i: (-(s3 + int(counts[pool[i]])))
                             % cfg.SLOTS)
                q.append(pool.pop(best_i))
            quads.append(q)
        assert not pool
        q_slabs = np.array([sum(int(counts[g]) for g in q) for q in quads])
        q_wins = np.ceil(q_slabs / cfg.SLOTS).astype(np.int64)
        q_wins = np.maximum(q_wins, 1)
        rank = np.argsort(-q_wins, kind="stable")
        per_core.append(([quads[j] for j in rank], q_wins[rank]))

    # common schedule: per rank, max window count over cores
    w_sched = np.max(np.stack([pc[1] for pc in per_core]), axis=0)
    cum_w = np.zeros(cfg.NQ + 1, dtype=np.int64)
    np.cumsum(w_sched, out=cum_w[1:])
    w_tot = int(cum_w[-1])

    # --- build per-core streams -------------------------------------------
    in_maps = []
    decode_quads = []
    for c in range(NC):
        quads_r, _ = per_core[c]
        slab_gene = np.full(w_tot * cfg.SLOTS, -1, dtype=np.int64)
        blk_ids, blk_pos, blk_rel = [], [], []
        for j in range(cfg.NQ):
            base = cum_w[j] * cfg.SLOTS
            p = 0
            for r, g in enumerate(quads_r[j]):
                s0, n = int(starts[g]), int(counts[g])
                ids = np.arange(s0, s0 + n)
                blk_ids.append(ids)
                blk_pos.append(base + p + np.arange(n))
                blk_rel.append(np.full(n, r, dtype=np.int64))
                p += n
            assert p <= int(w_sched[j]) * cfg.SLOTS
        blk_ids = np.concatenate(blk_ids)
        blk_pos = np.concatenate(blk_pos)
        blk_rel = np.concatenate(blk_rel)
        slab_gene[blk_pos] = src_s[blk_ids]

        # x slabs: [W, 128, BATCH] fp8
        xg = np.zeros((w_tot * cfg.SLOTS, B, BATCH), dtype=F8)
        m = slab_gene >= 0
        xg[m] = xslab8[slab_gene[m]]
        xg = xg.reshape(w_tot, cfg.SLOTS * B, BATCH)

        # scattered (pre-scaled) weights: [W, 128, 16] fp8
        wg5 = np.zeros((w_tot, cfg.SLOTS, B, cfg.QG, B), dtype=F8)
        wg5[blk_pos // cfg.SLOTS, blk_pos % cfg.SLOTS, :, blk_rel, :] = w_s8[blk_ids]
        wg = wg5.reshape(w_tot, cfg.SLOTS * B, cfg.QW)

        # combined stream, chunk-major along columns: chunk c of n windows is
        # a contiguous [128, n*PW] DRAM column block -> every DMA is a large
        # linear read (~440KB for full chunks).
        st = np.concatenate([xg, wg], axis=2)          # [W, 128, PW]
        sizes, cstarts = cfg.chunk_plan(w_tot)
        blocks = [
            st[cstarts[ci] : cstarts[ci] + n]
            .transpose(1, 0, 2).reshape(cfg.SLOTS * B, n * cfg.PW)
            for ci, n in enumerate(sizes)
        ]
        st = np.ascontiguousarray(np.concatenate(blocks, axis=1))

        # residual tiles: batch-major [128, NHB*256] e3m4; pair j's genes at
        # cols hb*256 + slot*16 + r*8 (mirrors the PSUM column layout)
        res = np.zeros((128, cfg.NHB * 256), dtype=F8)
        for j in range(cfg.NQ):
            hb, slot = j // 16, j % 16
            for r, g in enumerate(quads_r[j]):
                col = hb * 256 + slot * 16 + r * B
                res[:, col : col + B] = x8r[:, g * B : (g + 1) * B]

        in_maps.append({"st": st, "res": res})
        decode_quads.append(quads_r)

    return in_maps, w_sched, decode_quads


def _build_nc(cfg, w_sched):
    """Trace the (core-uniform) Bass program."""
    w_tot = int(np.sum(w_sched))
    PW = cfg.PW
    sizes, cstarts = cfg.chunk_plan(w_tot)
    nc = bacc.Bacc("TRN2")
    st = nc.dram_tensor("st", [128, w_tot * PW], mybir.dt.float8e3,
                        kind="ExternalInput")
    res = nc.dram_tensor("res", [128, cfg.NHB * 256], mybir.dt.float8e3,
                         kind="ExternalInput")
    out = nc.dram_tensor("out", [128, cfg.NHB * 256], mybir.dt.float16,
                         kind="ExternalOutput")

    cum_w = np.zeros(cfg.NQ + 1, dtype=np.int64)
    np.cumsum(w_sched, out=cum_w[1:])
    NW = cfg.BATCH            # x section width per window (128)

    with TileContext(nc) as tc:
        with (
            tc.tile_pool(name="stp", bufs=6) as stp,
            tc.tile_pool(name="psp", bufs=4, space="PSUM") as psp,
            tc.tile_pool(name="resp", bufs=8) as resp,
            tc.tile_pool(name="outp", bufs=4) as outp,
        ):
            st_t = None
            ci = -1                   # current chunk index
            k0 = 0                    # first window of current chunk
            res_t2 = None
            for hb in range(cfg.NHB):
                j0, j1 = hb * 16, min(hb * 16 + 16, cfg.NQ)
                if hb % 2 == 0:
                    # two half-banks per residual DMA: 512B/partition keeps
                    # the descriptor above the efficiency threshold
                    res_t2 = resp.tile([128, 512], mybir.dt.float8e3)
                    nc.gpsimd.dma_start(
                        out=res_t2, in_=res[:, hb * 256 : (hb + 2) * 256])
                res_t = res_t2[:, (hb % 2) * 256 : (hb % 2 + 1) * 256]
                ps = psp.tile([128, 256], mybir.dt.float32)
                for j in range(j0, j1):
                    c0 = cfg.QW * (j - j0)
                    t_first = int(cum_w[j])
                    t_last = int(cum_w[j + 1]) - 1
                    for t in range(t_first, t_last + 1):
                        if ci + 1 < len(sizes) and t == int(cstarts[ci + 1]):
                            ci += 1
                            k0 = int(cstarts[ci])
                            n = sizes[ci]
                            st_t = stp.tile([128, n * PW], mybir.dt.float8e3)
                            nc.sync.dma_start(
                                out=st_t[:, :],
                                in_=st[:, k0 * PW : (k0 + n) * PW])
                        k = t - k0
                        nc.tensor.matmul(
                            ps[:, c0 : c0 + cfg.QW],
                            st_t[:, k * PW : k * PW + NW],
                            st_t[:, k * PW + NW : (k + 1) * PW],
                            start=(t == t_first),
                            stop=(t == t_last),
                        )
                ot = outp.tile([128, 256], mybir.dt.float16)
                nc.vector.scalar_tensor_tensor(
                    ot, ps, 1.0 / WSCALE, res_t,
                    op0=mybir.AluOpType.mult, op1=mybir.AluOpType.add)
                out_eng = nc.sync if hb == cfg.NHB - 1 else nc.scalar
                out_eng.dma_start(out=out[:, hb * 256 : (hb + 1) * 256], in_=ot)
    if not nc.is_finalized():
        nc.finalize()
    return nc


def _decode(cfg, results, decode_quads):
    G, B, BATCH = cfg.G, cfg.B, cfg.BATCH
    full = np.empty((BATCH, G * B), dtype=np.float32)
    for c in range(cfg.NCORES):
        res = np.asarray(results[c]["out"], dtype=np.float32)
        for j in range(cfg.NQ):
            hb, slot = j // 16, j % 16
            for r, g in enumerate(decode_quads[c][j]):
                col = hb * 256 + slot * 16 + r * B
                full[:, g * B : (g + 1) * B] = res[:, col : col + B]
    return full


def _run(cfg, x, w, block_in, block_out, trace=False):
    in_maps, w_sched, decode_quads = _pack_host(cfg, x, w, block_in, block_out)
    nc = _build_nc(cfg, w_sched)
    r = run_bass_kernel_spmd(nc, in_maps, core_ids=list(range(cfg.NCORES)),
                             trace=trace)
    out = _decode(cfg, r.results, decode_quads)
    return out, r


def kernel(x, w, block_in, block_out):
    cfg = Cfg()
    out, _ = _run(cfg, x, w, block_in, block_out, trace=False)
    return out


# revision 24
# speedup vs baseline: 1.9470x; 1.0040x over previous
"""Trainium2 Bass kernel for nn_LinearPPI (block-sparse gene-gene message passing).

Computation (reference):
    out[b, 8*g_out + o] = sum_{n: block_out[n]=g_out} sum_i x[b, 8*block_in[n] + i] * w[n, i, o]
    out += x   (residual)

Strategy (v3, fp8 stream, batch-major PSUM):
  - Blocks sorted by destination gene; destination genes sharded over 8 cores
    (edge/expert parallel, no collectives needed).
  - Per core, genes are packed into PAIRS (QG=2).  Work is a stream of
    "windows": 16 x-slabs (one slab = 8 rows of x^T for one source gene =
    [8, 128]) stacked to a [128, 128] tile, plus a scattered weight tile
    [128, 16] (16 slabs x 8x8 block at the slab's gene-of-pair column).
  - The matmul is BATCH-MAJOR: the x window is the STATIONARY operand
    (lhsT, [K=128, M=128 batch]) and the weight tile is the MOVING operand
    (rhs, [K=128, N=16]).  One matmul per window:
        psum[0:128, c0:c0+16] (+)= x_win.T @ w_win   (K=128, M=128, N=16)
    Pair output regions are free-dim column ranges, so there is no PE
    32-partition quadrant constraint: QG=2 halves the zero-padding of the
    scattered weight tile vs QG=4 (50% vs 25% density), and PSUM banks pack
    densely (32 pairs x 16 cols = one [128, 512] bank; 250 pairs < 8 banks).
  - Both x and w stream in float8 E3M4 (4 mantissa bits).  Weights are
    pre-scaled by 32 on the host so they sit in the e3m4 normal range; the
    1/32 descale is fused into the combine.  Measured end-to-end relative
    error ~1.2e-2 vs the 2e-2 gate.
  - The residual is NOT in the stream: per half-bank (16 pairs = [128, 256])
    an fp16 tile holding the pairs' own-gene x columns (batch-major, so it is
    a direct column gather of x) is DMA'd in, and a single DVE
    scalar_tensor_tensor computes  out_sbuf = psum * (1/32) + residual,
    which is DMA'd to HBM as fp16.
  - The x-slab gather is done on the host (indices are known at trace time),
    producing a sequential HBM stream -> all device DMAs are large and
    contiguous (memory-bound regime; model DMA floor ~59us/core, achieved
    ~65us/core vs ~124us for the fp16 QG=2-quadrant baseline).
  - The per-core window schedule is made identical across cores (rank-sorted
    window-count maxima + zero-padding) so a single SPMD program serves all
    8 cores; per-core variation lives only in the streamed data.
  - Output is slot-ordered batch-major; the host inverse-permutes columns and
    concatenates shards.  No all-reduce: destination sharding makes each
    core's output disjoint.
"""

import math
import numpy as np
import ml_dtypes

import concourse.bacc as bacc
import concourse.mybir as mybir
from concourse.tile import TileContext
from concourse.bass_utils import run_bass_kernel_spmd

F8 = ml_dtypes.float8_e3m4
WSCALE = 32.0


class Cfg:
    def __init__(self, G=4000, B=8, BATCH=128, NCORES=8, chunk=24, qg=2):
        assert G % NCORES == 0
        self.G, self.B, self.BATCH, self.NCORES = G, B, BATCH, NCORES
        self.GPC = G // NCORES            # genes per core
        self.QG = qg                      # genes per pair
        assert self.GPC % self.QG == 0
        self.NQ = self.GPC // self.QG     # pairs per core (250)
        self.QW = self.QG * B             # psum cols per pair (16)
        self.NHB = math.ceil(self.NQ / 16)  # half-bank units of 16 pairs
        self.SLOTS = 16                   # slabs per window (K = 128)
        self.CH = chunk                   # windows per DMA chunk
        self.TAIL_CH = 4                  # chunk size for the last CH windows
        self.PW = BATCH + self.QW         # stream bytes/row/window (144)

    def chunk_plan(self, w_tot):
        """Chunk sizes: full CH chunks, then TAIL_CH-sized tail chunks so the
        final half-bank's compute tail after the last DMA is short."""
        sizes = []
        rem = w_tot
        while rem > self.CH:
            sizes.append(self.CH)
            rem -= self.CH
        while rem > 0:
            take = min(self.TAIL_CH, rem)
            sizes.append(take)
            rem -= take
        starts = np.zeros(len(sizes) + 1, dtype=np.int64)
        np.cumsum(sizes, out=starts[1:])
        return list(sizes), starts


def _pack_host(cfg, x, w, block_in, block_out):
    """Sort/shard/pad on the host. Returns (in_maps, w_sched, decode_quads)."""
    G, B, BATCH, NC = cfg.G, cfg.B, cfg.BATCH, cfg.NCORES

    src = np.asarray(block_in, dtype=np.int64)
    dst = np.asarray(block_out, dtype=np.int64)

    order = np.argsort(dst, kind="stable")
    src_s = src[order]
    w_s8 = np.ascontiguousarray(np.asarray(w, dtype=np.float32)[order] * WSCALE
                                ).astype(F8)
    counts = np.bincount(dst, minlength=G)
    starts = np.zeros(G + 1, dtype=np.int64)
    np.cumsum(counts, out=starts[1:])

    xf = np.asarray(x, dtype=np.float32)
    # x^T slabs: xslab[g] = x[:, 8g:8g+8].T  -> [G, 8, BATCH], fp8
    xslab8 = np.ascontiguousarray(xf.T.reshape(G, B, BATCH)).astype(F8)
    x8r = xf.astype(F8)                    # batch-major residual source

    # --- balanced gene->core assignment (snake over count-sorted genes) ---
    order_g = np.argsort(-counts, kind="stable")
    core_of = np.empty(G, dtype=np.int64)
    for r in range(0, G, 2 * NC):
        blk = order_g[r : r + 2 * NC]
        pat = list(range(NC)) + list(range(NC - 1, -1, -1))
        for i, g in enumerate(blk):
            core_of[g] = pat[i]

    # --- per-core pair packing: target sums that are multiples of SLOTS ---
    per_core = []
    for c in range(NC):
        genes = np.where(core_of == c)[0]  # this core's genes
        pool = sorted(genes.tolist(), key=lambda g: -counts[g])
        quads = []
        for _ in range(cfg.NQ):
            q = [pool.pop(0)]                       # largest remaining
            while pool and len(q) < cfg.QG - 1:     # middle picks: big/small mix
                q.append(pool.pop(0) if len(q) % 2 else pool.pop(-1))
            if pool and len(q) < cfg.QG:
                s3 = sum(int(counts[g]) for g in q)
                # last pick: minimize padding to the next multiple of SLOTS
                best_i = min(range(len(pool)),
                             key=lambda i: (-(s3 + int(counts[pool[i]])))
                             % cfg.SLOTS)
                q.append(pool.pop(best_i))
            quads.append(q)
        assert not pool
        q_slabs = np.array([sum(int(counts[g]) for g in q) for q in quads])
        q_wins = np.ceil(q_slabs / cfg.SLOTS).astype(np.int64)
        q_wins = np.maximum(q_wins, 1)
        rank = np.argsort(-q_wins, kind="stable")
        per_core.append(([quads[j] for j in rank], q_wins[rank]))

    # common schedule: per rank, max window count over cores
    w_sched = np.max(np.stack([pc[1] for pc in per_core]), axis=0)
    cum_w = np.zeros(cfg.NQ + 1, dtype=np.int64)
    np.cumsum(w_sched, out=cum_w[1:])
    w_tot = int(cum_w[-1])

    # --- build per-core streams -------------------------------------------
    in_maps = []
    decode_quads = []
    for c in range(NC):
        quads_r, _ = per_core[c]
        slab_gene = np.full(w_tot * cfg.SLOTS, -1, dtype=np.int64)
        blk_ids, blk_pos, blk_rel = [], [], []
        for j in range(cfg.NQ):
            base = cum_w[j] * cfg.SLOTS
            p = 0
            for r, g in enumerate(quads_r[j]):
                s0, n = int(starts[g]), int(counts[g])
                ids = np.arange(s0, s0 + n)
                blk_ids.append(ids)
                blk_pos.append(base + p + np.arange(n))
                blk_rel.append(np.full(n, r, dtype=np.int64))
                p += n
            assert p <= int(w_sched[j]) * cfg.SLOTS
        blk_ids = np.concatenate(blk_ids)
        blk_pos = np.concatenate(blk_pos)
        blk_rel = np.concatenate(blk_rel)
        slab_gene[blk_pos] = src_s[blk_ids]

        # x slabs: [W, 128, BATCH] fp8
        xg = np.zeros((w_tot * cfg.SLOTS, B, BATCH), dtype=F8)
        m = slab_gene >= 0
        xg[m] = xslab8[slab_gene[m]]
        xg = xg.reshape(w_tot, cfg.SLOTS * B, BATCH)

        # scattered (pre-scaled) weights: [W, 128, 16] fp8
        wg5 = np.zeros((w_tot, cfg.SLOTS, B, cfg.QG, B), dtype=F8)
        wg5[blk_pos // cfg.SLOTS, blk_pos % cfg.SLOTS, :, blk_rel, :] = w_s8[blk_ids]
        wg = wg5.reshape(w_tot, cfg.SLOTS * B, cfg.QW)

        # combined stream, chunk-major along columns: chunk c of n windows is
        # a contiguous [128, n*PW] DRAM column block -> every DMA is a large
        # linear read (~440KB for full chunks).
        st = np.concatenate([xg, wg], axis=2)          # [W, 128, PW]
        sizes, cstarts = cfg.chunk_plan(w_tot)
        blocks = [
            st[cstarts[ci] : cstarts[ci] + n]
            .transpose(1, 0, 2).reshape(cfg.SLOTS * B, n * cfg.PW)
            for ci, n in enumerate(sizes)
        ]
        st = np.ascontiguousarray(np.concatenate(blocks, axis=1))

        # residual tiles: batch-major [128, NHB*256] e3m4; pair j's genes at
        # cols hb*256 + slot*16 + r*8 (mirrors the PSUM column layout)
        res = np.zeros((128, cfg.NHB * 256), dtype=F8)
        for j in range(cfg.NQ):
            hb, slot = j // 16, j % 16
            for r, g in enumerate(quads_r[j]):
                col = hb * 256 + slot * 16 + r * B
                res[:, col : col + B] = x8r[:, g * B : (g + 1) * B]

        in_maps.append({"st": st, "res": res})
        decode_quads.append(quads_r)

    return in_maps, w_sched, decode_quads


def _build_nc(cfg, w_sched):
    """Trace the (core-uniform) Bass program."""
    w_tot = int(np.sum(w_sched))
    PW = cfg.PW
    sizes, cstarts = cfg.chunk_plan(w_tot)
    nc = bacc.Bacc("TRN2")
    st = nc.dram_tensor("st", [128, w_tot * PW], mybir.dt.float8e3,
                        kind="ExternalInput")
    res = nc.dram_tensor("res", [128, cfg.NHB * 256], mybir.dt.float8e3,
                         kind="ExternalInput")
    out = nc.dram_tensor("out", [128, cfg.NHB * 256], mybir.dt.float16,
                         kind="ExternalOutput")

    cum_w = np.zeros(cfg.NQ + 1, dtype=np.int64)
    np.cumsum(w_sched, out=cum_w[1:])
    NW = cfg.BATCH            # x section width per window (128)

    with TileContext(nc) as tc:
        with (
            tc.tile_pool(name="stp", bufs=6) as stp,
            tc.tile_pool(name="psp", bufs=4, space="PSUM") as psp,
            tc.tile_pool(name="resp", bufs=8) as resp,
            tc.tile_pool(name="outp", bufs=4) as outp,
        ):
            st_t = None
            ci = -1                   # current chunk index
            k0 = 0                    # first window of current chunk
            res_t2 = None
            for hb in range(cfg.NHB):
                j0, j1 = hb * 16, min(hb * 16 + 16, cfg.NQ)
                if hb % 2 == 0:
                    # two half-banks per residual DMA: 512B/partition keeps
                    # the descriptor above the efficiency threshold
                    res_t2 = resp.tile([128, 512], mybir.dt.float8e3)
                    nc.gpsimd.dma_start(
                        out=res_t2, in_=res[:, hb * 256 : (hb + 2) * 256])
                res_t = res_t2[:, (hb % 2) * 256 : (hb % 2 + 1) * 256]
                ps = psp.tile([128, 256], mybir.dt.float32)
                for j in range(j0, j1):
                    c0 = cfg.QW * (j - j0)
                    t_first = int(cum_w[j])
                    t_last = int(cum_w[j + 1]) - 1
                    for t in range(t_first, t_last + 1):
                        if ci + 1 < len(sizes) and t == int(cstarts[ci + 1]):
                            ci += 1
                            k0 = int(cstarts[ci])
                            n = sizes[ci]
                            st_t = stp.tile([128, n * PW], mybir.dt.float8e3)
                            nc.sync.dma_start(
                                out=st_t[:, :],
                                in_=st[:, k0 * PW : (k0 + n) * PW])
                        k = t - k0
                        nc.tensor.matmul(
                            ps[:, c0 : c0 + cfg.QW],
                            st_t[:, k * PW : k * PW + NW],
                            st_t[:, k * PW + NW : (k + 1) * PW],
                            start=(t == t_first),
                            stop=(t == t_last),
                        )
                ot = outp.tile([128, 256], mybir.dt.float16)
                nc.vector.scalar_tensor_tensor(
                    ot, ps, 1.0 / WSCALE, res_t,
                    op0=mybir.AluOpType.mult, op1=mybir.AluOpType.add)
                out_eng = nc.sync if hb == cfg.NHB - 1 else nc.scalar
                out_eng.dma_start(out=out[:, hb * 256 : (hb + 1) * 256], in_=ot)
    if not nc.is_finalized():
        nc.finalize()
    return nc


def _decode(cfg, results, decode_quads):
    G, B, BATCH = cfg.G, cfg.B, cfg.BATCH
    full = np.empty((BATCH, G * B), dtype=np.float32)
    for c in range(cfg.NCORES):
        res = np.asarray(results[c]["out"], dtype=np.float32)
        for j in range(cfg.NQ):
            hb, slot = j // 16, j % 16
            for r, g in enumerate(decode_quads[c][j]):
                col = hb * 256 + slot * 16 + r * B
                full[:, g * B : (g + 1) * B] = res[:, col : col + B]
    return full


def _run(cfg, x, w, block_in, block_out, trace=False):
    in_maps, w_sched, decode_quads = _pack_host(cfg, x, w, block_in, block_out)
    nc = _build_nc(cfg, w_sched)
    r = run_bass_kernel_spmd(nc, in_maps, core_ids=list(range(cfg.NCORES)),
                             trace=trace)
    out = _decode(cfg, r.results, decode_quads)
    return out, r


def kernel(x, w, block_in, block_out):
    cfg = Cfg()
    out, _ = _run(cfg, x, w, block_in, block_out, trace=False)
    return out
